# revision 1
# baseline (speedup 1.0000x reference)
"""DeepFusionCrossMamba Trainium2 kernel.

Launch-overhead-first design: through the axon tunnel each bound tensor
costs ~1.3 ms per call and each extra core in the mesh adds dispatch
bookkeeping, while the whole network is only ~3 ms of device work. So a
single core runs all 4 batches with fwd+bwd merged locally (no
collectives), and all inputs travel in ONE flat f32 blob parameter.

On-chip layout per batch: activations feature-major [feature(128), T];
the residual stream x is time-major [t_block(128), D]. The bwd direction
consumes a time-reversed copy of h made by a DRAM bounce + indirect row
gather (idx holds the reversed row permutation); its output is reversed
back the same way before the residual add.
"""

import numpy as np

import concourse.bass as bass
import concourse.bacc as bacc
import concourse.mybir as mybir
import concourse.tile as tile
from concourse.bass import IndirectOffsetOnAxis
from concourse.bass_utils import run_bass_kernel_spmd

F32 = mybir.dt.float32
BF16 = mybir.dt.bfloat16
I32 = mybir.dt.int32
AF = mybir.ActivationFunctionType
OP = mybir.AluOpType

B, T, D = 4, 1024, 256
NM, DI, DS, DCONV, DTR = 2, 512, 16, 4, 16
NDCH = D // 128    # 2 feature chunks of d_model
NICH = DI // 128   # 4 feature chunks of d_inner
NTB = T // 128     # 8 time blocks
SP = 2             # s-values packed per scan instruction
EPS = 1e-8
NBPC = 4           # batches per core (all of them; single-core mesh)

_CACHE = {}

# Single-blob input packing. Order defines offsets on host and device.
# idx is int32 bit-packed (bitcast on device).
PACK = [
    ("a_fm", [NBPC, D, T], "h"),
    ("v_fm", [NBPC, D, T], "h"),
    ("w_a2v", [D, D], "h"),
    ("b_a2v", [128, NDCH], "f"),
    ("w_v2a", [D, D], "h"),
    ("b_v2a", [128, NDCH], "f"),
    ("pk", [3, D, D], "h"),
    ("bn_s", [128, NDCH], "f"),
    ("bn_b", [128, NDCH], "f"),
    ("rmsw_bc", [NM, 128, D], "f"),
    ("w_in", [NM, 2, D, 2 * DI], "h"),
    ("cw", [NM, 2, 128, NICH * DCONV], "f"),
    ("cb", [NM, 2, 128, NICH], "f"),
    ("w_xp", [NM, 2, DI, 80], "h"),
    ("w_dt", [NM, 2, DTR, DI], "h"),
    ("dtb", [NM, 2, 128, NICH], "f"),
    ("a_neg", [NM, 2, 128, NICH * DS], "f"),
    ("dsk", [NM, 2, 128, NICH], "f"),
    ("w_out", [NM, 2, DI, D], "h"),
    ("g_bc", [128, D], "f"),
    ("be_bc", [128, D], "f"),
    ("id128", [128, 128], "f"),
    ("idx", [128, NTB], "i"),
]


def _pack_offsets():
    offs, off = {}, 0
    for name, shape, dt in PACK:
        n = 1
        for s in shape:
            n *= s
        if dt == "h":
            assert n % 2 == 0
            n //= 2
        offs[name] = (off, list(shape), dt)
        off += n
    return offs, off


OFFS, BLOB_N = _pack_offsets()


def _decl(nc, name, shape, dtype=F32, out=False):
    return nc.declare_dram_parameter(name, list(shape), dtype, isOutput=out)


def build_nc(nlayers=NM, nbatches=NBPC, probe=(), sp=SP, dab=False):
    nc = bacc.Bacc(None, target_bir_lowering=False, debug=False)
    blob_d = _decl(nc, "blob", [BLOB_N])
    out_d = _decl(nc, "out", [NBPC * T, D], BF16, out=True)
    with tile.TileContext(nc) as tc:
        _body(nc, tc, nlayers, nbatches, blob_d, out_d, probe, sp, dab)
    nc.finalize()
    return nc


def _body(nc, tc, nlayers, nbatches, blob_d, out_d, probe=(), sp=SP,
          dab=False):
    from contextlib import ExitStack
    ctx = ExitStack()
    with ctx:
        perm = ctx.enter_context(tc.tile_pool(name="perm", bufs=1))
        pwork = ctx.enter_context(tc.tile_pool(name="pwork", bufs=1))
        pscan = ctx.enter_context(tc.tile_pool(name="pscan", bufs=1))
        phc = ctx.enter_context(tc.tile_pool(name="phc", bufs=2 if sp == 2 else 1))
        pdiag = ctx.enter_context(tc.tile_pool(name="pdiag", bufs=1))
        psmall = ctx.enter_context(tc.tile_pool(name="psmall", bufs=2))
        ppsum = ctx.enter_context(tc.tile_pool(name="ppsum", bufs=6, space="PSUM"))
        ppsy = ctx.enter_context(tc.tile_pool(name="ppsy", bufs=2, space="PSUM"))

        th_sl = [slice(0, 512), slice(512, 1024)]

        blob_ap = blob_d[:]

        def bsl(name, *pre, rows=None):
            off, shape, dt = OFFS[name]
            div = 2 if dt == "h" else 1
            for i, ix in enumerate(pre):
                stride = 1
                for s in shape[i + 1:]:
                    stride *= s
                off += ix * stride // div
            s = shape[len(pre):]
            assert len(s) == 2
            r0, r1 = (0, s[0]) if rows is None else rows
            off += r0 * s[1] // div
            n = (r1 - r0) * s[1] // div
            ap = blob_ap[off:off + n].rearrange("(a b) -> a b", a=r1 - r0)
            if dt == "h":
                ap = ap.bitcast(BF16)
            elif dt == "i":
                ap = ap.bitcast(I32)
            return ap

        def load(dram, shape, name, dtype=F32, pool=perm, eng=None):
            if not isinstance(dram, bass.AP):
                dram = dram[:]
            t = pool.tile(shape, dtype, tag=name)
            (eng or nc.sync).dma_start(t[:], dram)
            return t

        # ---------------- shared persistent loads ----------------
        w_a2v = [load(bsl("w_a2v", rows=(c * 128, (c + 1) * 128)), [128, D],
                      f"w_a2v{c}", BF16) for c in range(NDCH)]
        w_v2a = [load(bsl("w_v2a", rows=(c * 128, (c + 1) * 128)), [128, D],
                      f"w_v2a{c}", BF16) for c in range(NDCH)]
        b_a2v = load(bsl("b_a2v"), [128, NDCH], "b_a2v")
        b_v2a = load(bsl("b_v2a"), [128, NDCH], "b_v2a")
        pk = [[load(bsl("pk", k, rows=(c * 128, (c + 1) * 128)), [128, D],
                    f"pk{k}{c}", BF16)
               for c in range(NDCH)] for k in range(3)]
        bn_s = load(bsl("bn_s"), [128, NDCH], "bn_s")
        bn_b = load(bsl("bn_b"), [128, NDCH], "bn_b")
        g_bc = load(bsl("g_bc"), [128, D], "g_bc")
        be_bc = load(bsl("be_bc"), [128, D], "be_bc")
        id128 = load(bsl("id128"), [128, 128], "id128")
        idx = load(bsl("idx"), [128, NTB], "idx", I32)
        id128b = perm.tile([128, 128], BF16, tag="id128b", name="id128b")
        nc.vector.tensor_copy(out=id128b[:], in_=id128[:])
        rmsw = [load(bsl("rmsw_bc", l), [128, D], f"rmsw{l}")
                for l in range(nlayers)]

        bounceY = [nc.dram_tensor(f"bounceY{i}", [T, D], BF16) for i in range(2)]
        xdbl_dram = nc.dram_tensor("xdbl_dram", [32, T], BF16)

        for bi in range(nbatches):
            # ---------------- preamble (feature-major) ----------------
            a_fm = [load(bsl("a_fm", bi, rows=(c * 128, (c + 1) * 128)), [128, T],
                         f"a_fm{c}", BF16, pool=pwork) for c in range(NDCH)]
            a_tm = perm.tile([128, NTB * D], F32, tag="a_tm", name="a_tm")
            for j in range(NTB):
                for dc in range(NDCH):
                    psT = ppsum.tile([128, 128], F32, tag="ps", name="ps")
                    nc.tensor.matmul(psT[:], a_fm[dc][:, j * 128:(j + 1) * 128],
                                     id128b[:], start=True, stop=True)
                    nc.scalar.copy(a_tm[:, j * D + dc * 128: j * D + (dc + 1) * 128],
                                   psT[:])
            g_a2v = [pwork.tile([128, T], F32, tag=f"g_z{c}", name=f"g_a2v{c}")
                     for c in range(NDCH)]
            for ec in range(NDCH):
                for th in range(2):
                    ps = ppsum.tile([128, 512], F32, tag="ps", name="ps")
                    for dc in range(NDCH):
                        nc.tensor.matmul(ps[:], w_a2v[dc][:, ec * 128:(ec + 1) * 128],
                                         a_fm[dc][:, th_sl[th]],
                                         start=(dc == 0), stop=(dc == NDCH - 1))
                    nc.scalar.activation(g_a2v[ec][:, th_sl[th]], ps[:], AF.Sigmoid,
                                         bias=b_a2v[:, ec:ec + 1])
            # v_ref, padded for the 3-tap conv: col j holds v_ref[t=j-1]
            v_fm = [load(bsl("v_fm", bi, rows=(c * 128, (c + 1) * 128)), [128, T],
                         f"v_fm{c}", BF16, pool=pwork) for c in range(NDCH)]
            v_pad = [pwork.tile([128, T + 3], BF16, tag=f"v_pad{c}", name=f"v_pad{c}")
                     for c in range(NDCH)]
            for c in range(NDCH):
                nc.vector.memset(v_pad[c][:, 0:1], 0.0)
                nc.vector.memset(v_pad[c][:, T + 1:T + 2], 0.0)
                nc.gpsimd.tensor_tensor(out=v_pad[c][:, 1:T + 1], in0=v_fm[c][:],
                                        in1=g_a2v[c][:], op=OP.mult)

            g_v2a = [pwork.tile([128, T], F32, tag=f"g_z{c+2}", name=f"g_v2a{c}")
                     for c in range(NDCH)]
            for ec in range(NDCH):
                for th in range(2):
                    ps = ppsum.tile([128, 512], F32, tag="ps", name="ps")
                    for dc in range(NDCH):
                        nc.tensor.matmul(ps[:], w_v2a[dc][:, ec * 128:(ec + 1) * 128],
                                         v_pad[dc][:, th * 512 + 1: th * 512 + 513],
                                         start=(dc == 0), stop=(dc == NDCH - 1))
                    nc.scalar.activation(g_v2a[ec][:, th_sl[th]], ps[:], AF.Sigmoid,
                                         bias=b_v2a[:, ec:ec + 1])
            dlt = [pwork.tile([128, T], F32, tag=f"xc_s{c+2}", name=f"dlt{c}")
                   for c in range(NDCH)]
            for ec in range(NDCH):
                for th in range(2):
                    ps = ppsum.tile([128, 512], F32, tag="ps", name="ps")
                    first = True
                    for k in range(3):
                        for dc in range(NDCH):
                            nc.tensor.matmul(
                                ps[:], pk[k][dc][:, ec * 128:(ec + 1) * 128],
                                v_pad[dc][:, th * 512 + k: th * 512 + k + 512],
                                start=first, stop=(k == 2 and dc == NDCH - 1))
                            first = False
                    nc.scalar.activation(dlt[ec][:, th_sl[th]], ps[:], AF.Gelu,
                                         bias=bn_b[:, ec:ec + 1],
                                         scale=bn_s[:, ec:ec + 1])
            gdlt = [pwork.tile([128, T], F32, tag=f"xc_pad{c}", name=f"gdlt{c}")
                    for c in range(NDCH)]
            for c in range(NDCH):
                nc.gpsimd.tensor_tensor(out=gdlt[c][:], in0=g_v2a[c][:],
                                        in1=dlt[c][:], op=OP.mult)
            # x0 (time-major) = a_tm + transpose(gdlt)
            x_tm = perm.tile([128, NTB * D], F32, tag="x_tm0", name="x_tm0")
            for j in range(NTB):
                for dc in range(NDCH):
                    psT = ppsum.tile([128, 128], F32, tag="ps", name="ps")
                    nc.tensor.transpose(psT[:], gdlt[dc][:, j * 128:(j + 1) * 128],
                                        id128[:])
                    sl = slice(j * D + dc * 128, j * D + (dc + 1) * 128)
                    nc.vector.tensor_tensor(out=x_tm[:, sl], in0=a_tm[:, sl],
                                            in1=psT[:], op=OP.add)

            # ---------------- mamba layers ----------------
            for l in range(nlayers):
                # rmsnorm stats over channel dim (free dim in TM layout)
                st = psmall.tile([128, NTB], F32, tag="st", name="st")
                sq = pwork.tile([128, D], F32, tag="sq", name="sq")
                for j in range(NTB):
                    nc.scalar.activation(sq[:], x_tm[:, j * D:(j + 1) * D],
                                         AF.Square, accum_out=st[:, j:j + 1])
                ms = psmall.tile([128, NTB], F32, tag="ms", name="ms")
                nc.vector.tensor_scalar(out=ms[:], in0=st[:], scalar1=1.0 / D,
                                        scalar2=1e-5, op0=OP.mult, op1=OP.add)
                msr = psmall.tile([128, NTB], F32, tag="msr", name="msr")
                nc.vector.reciprocal(out=msr[:], in_=ms[:])
                rstd = psmall.tile([128, NTB], F32, tag="rstd", name="rstd")
                nc.scalar.activation(rstd[:], msr[:], AF.Sqrt)

                h_tm = pwork.tile([128, NTB * D], F32, tag="h_tm", name="h_tm")
                for j in range(NTB):
                    nc.vector.scalar_tensor_tensor(
                        out=h_tm[:, j * D:(j + 1) * D],
                        in0=x_tm[:, j * D:(j + 1) * D],
                        scalar=rstd[:, j:j + 1], in1=rmsw[l][:],
                        op0=OP.mult, op1=OP.mult)
                h_fm = [pwork.tile([128, T], BF16, tag=f"h_fm{c}", name=f"h_fm{c}")
                        for c in range(NDCH)]
                for j in range(NTB):
                    for dc in range(NDCH):
                        psT = ppsum.tile([128, 128], F32, tag="ps", name="ps")
                        nc.tensor.transpose(
                            psT[:], h_tm[:, j * D + dc * 128: j * D + (dc + 1) * 128],
                            id128[:])
                        nc.scalar.copy(h_fm[dc][:, j * 128:(j + 1) * 128], psT[:])
                # bwd branch consumes h with the time (free) dim reversed —
                # the PE moving operand reads a negative-stride AP directly
                h_rev_fm = [h_fm[c][:, ::-1] for c in range(NDCH)]

                ytm = [None, None]
                for r in range(2):
                    hsrc = h_fm if r == 0 else h_rev_fm
                    w_in_l = [load(bsl("w_in", l, r, rows=(c * 128, (c + 1) * 128)),
                                   [128, 2 * DI], f"w_in{c}", BF16, eng=nc.scalar)
                              for c in range(NDCH)]
                    cw_l = load(bsl("cw", l, r), [128, NICH * DCONV], "cw")
                    cb_l = load(bsl("cb", l, r), [128, NICH], "cb")
                    w_xp_l = [load(bsl("w_xp", l, r, rows=(c * 128, (c + 1) * 128)),
                                   [128, 80], f"w_xp{c}", BF16) for c in range(NICH)]
                    w_dt_l = load(bsl("w_dt", l, r), [DTR, DI], "w_dt", BF16)
                    dtb_l = load(bsl("dtb", l, r), [128, NICH], "dtb")
                    a_neg_l = load(bsl("a_neg", l, r), [128, NICH * DS], "a_neg")
                    dsk_l = load(bsl("dsk", l, r), [128, NICH], "dsk")
                    w_out_l = [load(bsl("w_out", l, r, rows=(c * 128, (c + 1) * 128)),
                                    [128, D], f"w_out{c}", BF16, eng=nc.scalar)
                               for c in range(NICH)]

                    # in_proj -> xc (padded), silu(z)
                    xc_pad = [pwork.tile([128, T + 3], F32, tag=f"xc_pad{c}",
                                         name=f"xc_pad{c}") for c in range(NICH)]
                    g_z = [pwork.tile([128, T], F32, tag=f"g_z{c}", name=f"g_z{c}")
                           for c in range(NICH)]
                    for c in range(NICH):
                        nc.vector.memset(xc_pad[c][:, 0:3], 0.0)
                    for ec in range(2 * NICH):
                        for th in range(2):
                            ps = ppsum.tile([128, 512], F32, tag="ps", name="ps")
                            for dc in range(NDCH):
                                nc.tensor.matmul(
                                    ps[:], w_in_l[dc][:, ec * 128:(ec + 1) * 128],
                                    hsrc[dc][:, th_sl[th]],
                                    start=(dc == 0), stop=(dc == NDCH - 1))
                            if ec < NICH:
                                nc.scalar.copy(
                                    xc_pad[ec][:, 3 + th * 512: 3 + th * 512 + 512],
                                    ps[:])
                            else:
                                nc.scalar.activation(g_z[ec - NICH][:, th_sl[th]],
                                                     ps[:], AF.Silu)
                    # depthwise causal conv: DVE tensor_scalar/stt chain + silu
                    xc_s = [pwork.tile([128, T], BF16, tag=f"xc_s{c}",
                                       name=f"xc_s{c}") for c in range(NICH)]
                    cvo = pwork.tile([128, T], F32, tag="delta1", name="cvo")
                    cvp = pwork.tile([128, T], F32, tag="esp", name="cvp")
                    for c in range(NICH):
                        acc = [cvo, cvp]
                        nc.vector.tensor_scalar(acc[0][:], xc_pad[c][:, 0:T],
                                                cw_l[:, c * DCONV:c * DCONV + 1],
                                                None, OP.mult)
                        for k in range(1, DCONV):
                            nc.vector.scalar_tensor_tensor(
                                out=acc[k % 2][:], in0=xc_pad[c][:, k:k + T],
                                scalar=cw_l[:, c * DCONV + k:c * DCONV + k + 1],
                                in1=acc[(k + 1) % 2][:], op0=OP.mult, op1=OP.add)
                        nc.scalar.activation(xc_s[c][:], acc[(DCONV - 1) % 2][:],
                                             AF.Silu, bias=cb_l[:, c:c + 1])
                    # x_proj -> xdbl rows [dt(16) | B(16) | C(16)]
                    xdbl = pwork.tile([DTR, T], BF16, tag="xdbl", name="xdbl")
                    xdbl_bf = pwork.tile([48, T], BF16, tag="xdbl_bf", name="xdbl_bf")
                    for th in range(2):
                        psx = ppsum.tile([80, 512], F32, tag="ps", name="ps")
                        for c in range(NICH):
                            nc.tensor.matmul(psx[:], w_xp_l[c][:],
                                             xc_s[c][:, th_sl[th]],
                                             start=(c == 0), stop=(c == NICH - 1))
                        nc.scalar.copy(xdbl[0:DTR, th_sl[th]], psx[0:DTR, :])
                        nc.scalar.copy(xdbl_bf[0:16, th_sl[th]], psx[32:48, :])
                        nc.scalar.copy(xdbl_bf[32:48, th_sl[th]], psx[64:80, :])
                        nc.sync.dma_start(xdbl_dram[0:16, th_sl[th]],
                                          xdbl_bf[0:16, th_sl[th]])
                        nc.sync.dma_start(xdbl_dram[16:32, th_sl[th]],
                                          xdbl_bf[32:48, th_sl[th]])

                    # per-chunk: delta, u, scan over (s,t), y accumulation
                    y_g = [pwork.tile([128, T], BF16, tag=f"y_g{c}",
                                      name=f"y_g{c}") for c in range(NICH)]
                    nsb = 2 if sp == 2 else 1
                    dadt = BF16 if dab else F32
                    dA2 = [pscan.tile([128, sp * T], dadt, tag=f"dA{i}",
                                      name=f"dA{i}") for i in range(nsb)]
                    dBu2 = [pscan.tile([128, sp * T], BF16, tag=f"dBu{i}",
                                       name=f"dBu{i}") for i in range(nsb)]
                    hsc = pscan.tile([128, sp * T], BF16, tag="hsc", name="hsc")
                    for i in range(nsb):
                        nc.vector.memset(dA2[i][:, 0:sp * T:T], 0.0)
                    # softplus + u prep, software-pipelined one chunk ahead
                    # so the next chunk's scan inputs are ready the moment
                    # the current chunk's scans drain
                    du = [None] * NICH

                    def prep(c):
                        psd = [ppsum.tile([128, 512], F32, tag="ps", name="ps")
                               for _ in range(2)]
                        for th in range(2):
                            nc.tensor.matmul(psd[th][:],
                                             w_dt_l[:, c * 128:(c + 1) * 128],
                                             xdbl[0:DTR, th_sl[th]],
                                             start=True, stop=True)
                        delta = pwork.tile([128, T], F32, tag=f"delta{c % 2}",
                                           name="delta")
                        esp = pwork.tile([128, T], F32, tag=f"esp{c % 2}",
                                         name="esp")
                        for th in range(2):
                            # softplus(x+b) = ln(1+exp(x+b)); exp & ln share a table
                            nc.scalar.activation(esp[:, th_sl[th]], psd[th][:],
                                                 AF.Exp, bias=dtb_l[:, c:c + 1])
                        for th in range(2):
                            nc.scalar.activation(delta[:, th_sl[th]],
                                                 esp[:, th_sl[th]], AF.Ln,
                                                 bias=1.0)
                        u = pwork.tile([128, T], BF16, tag=f"u{c % 2}", name="u")
                        nc.gpsimd.tensor_tensor(out=u[:], in0=delta[:],
                                                in1=xc_s[c][:], op=OP.mult)
                        du[c] = (delta, u)

                    # skip-connection diagonals built up front, off the
                    # per-chunk critical-path tail
                    dgds = []
                    for c in range(NICH):
                        dgd = pdiag.tile([128, 128], BF16, tag=f"cdiag{c}",
                                         name=f"cdiag{c}")
                        nc.vector.tensor_scalar(out=dgd[:], in0=id128b[:],
                                                scalar1=dsk_l[:, c:c + 1],
                                                scalar2=None, op0=OP.mult)
                        dgds.append(dgd)

                    prep(0)
                    for c in range(NICH):
                        if c + 1 < NICH:
                            prep(c + 1)
                        delta, u = du[c]

                        psy = [ppsy.tile([128, 512], F32, tag="psy", name="psy")
                               for _ in range(2)]
                        nsp = DS // sp
                        for spi in range(nsp):
                            dA = dA2[spi % nsb]
                            dBu = dBu2[spi % nsb]
                            for si in range(sp):
                                sv = spi * sp + si
                                nc.scalar.activation(
                                    dA[:, si * T + 1:(si + 1) * T],
                                    delta[:, 1:T], AF.Exp,
                                    scale=a_neg_l[:, c * DS + sv:
                                                  c * DS + sv + 1])
                            s0 = spi * sp
                            bm = phc.tile([128, sp * T], BF16, tag="bm", name="bm")
                            if "no_bcast" in probe:
                                nc.sync.dma_start(bm[0:sp, 0:T], xdbl_dram[s0:s0 + sp, :])
                            else:
                                nc.sync.dma_start(
                                    bm[:], xdbl_dram[s0:s0 + sp, :]
                                    .rearrange("a b -> (a b)").partition_broadcast(128))
                            for si in range(sp):
                                nc.gpsimd.tensor_tensor(
                                    out=dBu[:, si * T:(si + 1) * T], in0=u[:],
                                    in1=bm[:, si * T:(si + 1) * T], op=OP.mult)
                            cm = phc.tile([128, sp * T], BF16, tag="cm", name="cm")
                            if "no_bcast" in probe:
                                nc.sync.dma_start(cm[0:sp, 0:T],
                                                  xdbl_dram[16 + s0:16 + s0 + sp, :])
                            else:
                                nc.sync.dma_start(
                                    cm[:], xdbl_dram[16 + s0:16 + s0 + sp, :]
                                    .rearrange("a b -> (a b)").partition_broadcast(128))
                            hc = phc.tile([128, sp * T], BF16, tag="hc", name="hc")
                            if "no_scan" in probe:
                                nc.gpsimd.tensor_tensor(
                                    out=hc[:], in0=dBu[:], in1=cm[:], op=OP.mult)
                            else:
                                nc.vector.tensor_tensor_scan(
                                    hsc[:], dA[:], dBu[:], 0.0, OP.mult, OP.add)
                                nc.vector.tensor_tensor(
                                    out=hc[:], in0=hsc[:], in1=cm[:], op=OP.mult)
                            for si in range(sp):
                                for th in range(2):
                                    nc.tensor.matmul(
                                        psy[th][:], id128b[:],
                                        hc[:, si * T + th * 512: si * T + th * 512 + 512],
                                        start=(spi == 0 and si == 0), stop=False)
                        # skip connection D_skip * xc
                        for th in range(2):
                            nc.tensor.matmul(psy[th][:], dgds[c][:],
                                             xc_s[c][:, th_sl[th]],
                                             start=False, stop=True)
                        for th in range(2):
                            nc.vector.tensor_tensor(out=y_g[c][:, th_sl[th]],
                                                    in0=psy[th][:],
                                                    in1=g_z[c][:, th_sl[th]],
                                                    op=OP.mult)

                    # out_proj, time-major output blocks
                    ytag = "h_tm" if r == 0 else "h_rev"
                    ytm[r] = pwork.tile([128, NTB * D], BF16, tag=ytag,
                                        name=f"ytm{r}")
                    for j in range(NTB):
                        pso = ppsum.tile([128, D], F32, tag="ps", name="ps")
                        for c in range(NICH):
                            nc.tensor.matmul(pso[:], y_g[c][:, j * 128:(j + 1) * 128],
                                             w_out_l[c][:],
                                             start=(c == 0), stop=(c == NICH - 1))
                        nc.scalar.copy(ytm[r][:, j * D:(j + 1) * D], pso[:])
                    if r == 0:
                        # fwd residual add now — its inputs are ready, and the
                        # Pool engine absorbs it under the bwd branch's scans
                        x_new = perm.tile([128, NTB * D], F32,
                                          tag=f"x_tm{(l + 1) % 2}",
                                          name=f"x_tm{(l + 1) % 2}")
                        for j in range(NTB):
                            sl = slice(j * D, (j + 1) * D)
                            nc.gpsimd.tensor_tensor(out=x_new[:, sl],
                                                    in0=x_tm[:, sl],
                                                    in1=ytm[0][:, sl], op=OP.add)

                # reverse bwd output back to natural frame via bounce
                by = bounceY[l % 2]
                for j in range(NTB):
                    nc.sync.dma_start(by[j * 128:(j + 1) * 128, :],
                                      ytm[1][:, j * D:(j + 1) * D])
                artm = pwork.tile([128, NTB * D], BF16, tag="h_fm0", name="artm")
                for j in range(NTB):
                    nc.gpsimd.indirect_dma_start(
                        out=artm[:, j * D:(j + 1) * D], out_offset=None,
                        in_=by[:],
                        in_offset=IndirectOffsetOnAxis(ap=idx[:, j:j + 1], axis=0))
                for j in range(NTB):
                    sl = slice(j * D, (j + 1) * D)
                    nc.gpsimd.tensor_tensor(out=x_new[:, sl], in0=x_new[:, sl],
                                            in1=artm[:, sl], op=OP.add)
                x_tm = x_new

            # ---------------- final channel LayerNorm ----------------
            s_t = pwork.tile([128, NTB * D], F32, tag="h_tm", name="s_t")
            nc.gpsimd.tensor_tensor(out=s_t[:], in0=x_tm[:], in1=a_tm[:], op=OP.add)
            stm = psmall.tile([128, NTB], F32, tag="stm", name="stm")
            stv = psmall.tile([128, NTB], F32, tag="stv", name="stv")
            dump = pwork.tile([128, D], F32, tag="sq", name="sq")
            for j in range(NTB):
                nc.scalar.activation(dump[:], s_t[:, j * D:(j + 1) * D], AF.Copy,
                                     accum_out=stm[:, j:j + 1])
                nc.scalar.activation(dump[:], s_t[:, j * D:(j + 1) * D], AF.Square,
                                     accum_out=stv[:, j:j + 1])
            mu = psmall.tile([128, NTB], F32, tag="mu", name="mu")
            nc.vector.tensor_scalar(out=mu[:], in0=stm[:], scalar1=1.0 / D,
                                    scalar2=None, op0=OP.mult)
            var = psmall.tile([128, NTB], F32, tag="var", name="var")
            nc.vector.tensor_scalar(out=var[:], in0=stv[:], scalar1=1.0 / D,
                                    scalar2=None, op0=OP.mult)
            mu2 = psmall.tile([128, NTB], F32, tag="mu2", name="mu2")
            nc.vector.tensor_tensor(out=mu2[:], in0=mu[:], in1=mu[:], op=OP.mult)
            nc.vector.tensor_tensor(out=var[:], in0=var[:], in1=mu2[:],
                                    op=OP.subtract)
            ve = psmall.tile([128, NTB], F32, tag="ve", name="ve")
            nc.vector.tensor_scalar(out=ve[:], in0=var[:], scalar1=EPS,
                                    scalar2=None, op0=OP.add)
            vr = psmall.tile([128, NTB], F32, tag="vr", name="vr")
            nc.vector.reciprocal(out=vr[:], in_=ve[:])
            rstd2 = psmall.tile([128, NTB], F32, tag="rstd2", name="rstd2")
            nc.scalar.activation(rstd2[:], vr[:], AF.Sqrt)
            otm = pwork.tile([128, NTB * D], BF16, tag="h_fm0", name="otm")
            for j in range(NTB):
                sl = slice(j * D, (j + 1) * D)
                nc.vector.tensor_scalar(out=otm[:, sl], in0=s_t[:, sl],
                                        scalar1=mu[:, j:j + 1],
                                        scalar2=rstd2[:, j:j + 1],
                                        op0=OP.subtract, op1=OP.mult)
                nc.vector.tensor_tensor(out=otm[:, sl], in0=otm[:, sl],
                                        in1=g_bc[:], op=OP.mult)
                nc.vector.tensor_tensor(out=otm[:, sl], in0=otm[:, sl],
                                        in1=be_bc[:], op=OP.add)
            for j in range(NTB):
                nc.sync.dma_start(
                    out_d[(bi * NTB + j) * 128:(bi * NTB + j + 1) * 128, :],
                    otm[:, j * D:(j + 1) * D])


# ---------------- host side ----------------

def make_in_maps(inputs, num_cores=1):
    inp = {k: np.asarray(v, dtype=np.float32) for k, v in inputs.items()}
    m = {}
    m["a_fm"] = np.ascontiguousarray(inp["audio"].transpose(0, 2, 1))
    m["v_fm"] = np.ascontiguousarray(inp["video"].transpose(0, 2, 1))
    m["w_a2v"] = np.ascontiguousarray(inp["gate_a2v_w"].T)
    m["b_a2v"] = np.ascontiguousarray(inp["gate_a2v_b"].reshape(NDCH, 128).T)
    m["w_v2a"] = np.ascontiguousarray(inp["gate_v2a_w"].T)
    m["b_v2a"] = np.ascontiguousarray(inp["gate_v2a_b"].reshape(NDCH, 128).T)
    m["pk"] = np.ascontiguousarray(
        np.stack([inp["proj_w"][:, :, k].T for k in range(3)]))
    m["bn_s"] = np.ascontiguousarray(
        (inp["bn_gamma"] / np.sqrt(1.0 + 1e-5)).reshape(NDCH, 128).T)
    m["bn_b"] = np.ascontiguousarray(inp["bn_beta"].reshape(NDCH, 128).T)
    m["rmsw_bc"] = np.ascontiguousarray(
        np.broadcast_to(inp["rms_w"][:, None, :], (NM, 128, D)))
    m["w_in"] = np.ascontiguousarray(inp["in_proj_w"].transpose(0, 1, 3, 2))
    m["cw"] = np.ascontiguousarray(
        inp["conv_w"].reshape(NM, 2, NICH, 128, DCONV)
        .transpose(0, 1, 3, 2, 4).reshape(NM, 2, 128, NICH * DCONV))
    m["cb"] = np.ascontiguousarray(
        inp["conv_b"].reshape(NM, 2, NICH, 128).transpose(0, 1, 3, 2))
    w_xp_p = np.zeros((NM, 2, DI, 80), np.float32)
    for l in range(NM):
        for r in range(2):
            xp_t = inp["x_proj_w"][l, r].T  # [DI, 48]
            w_xp_p[l, r, :, 0:DTR] = xp_t[:, 0:DTR]
            w_xp_p[l, r, :, 32:32 + DS] = xp_t[:, DTR:DTR + DS]
            w_xp_p[l, r, :, 64:64 + DS] = xp_t[:, DTR + DS:DTR + 2 * DS]
    m["w_xp"] = w_xp_p
    m["w_dt"] = np.ascontiguousarray(inp["dt_w"].transpose(0, 1, 3, 2))
    m["dtb"] = np.ascontiguousarray(
        inp["dt_b"].reshape(NM, 2, NICH, 128).transpose(0, 1, 3, 2))
    m["a_neg"] = np.ascontiguousarray(
        (-np.exp(inp["A_log"])).reshape(NM, 2, NICH, 128, DS)
        .transpose(0, 1, 3, 2, 4).reshape(NM, 2, 128, NICH * DS))
    m["dsk"] = np.ascontiguousarray(
        inp["D_skip"].reshape(NM, 2, NICH, 128).transpose(0, 1, 3, 2))
    m["w_out"] = np.ascontiguousarray(inp["out_w"].transpose(0, 1, 3, 2))
    m["g_bc"] = np.ascontiguousarray(np.broadcast_to(inp["cln_gamma"], (128, D)))
    m["be_bc"] = np.ascontiguousarray(np.broadcast_to(inp["cln_beta"], (128, D)))
    m["id128"] = np.eye(128, dtype=np.float32)
    t_of = np.arange(T, dtype=np.int64).reshape(NTB, 128).T  # [128, NTB]
    m["idx"] = np.ascontiguousarray(((T - 1) - t_of).astype(np.int32))

    import ml_dtypes
    parts = []
    for name, shape, dt in PACK:
        a = np.ascontiguousarray(m[name])
        assert a.shape == tuple(shape), (name, a.shape, shape)
        if dt == "h":
            a = a.astype(ml_dtypes.bfloat16).ravel().view(np.float32)
        elif dt == "i":
            a = a.view(np.float32).ravel()
        else:
            a = a.ravel()
        parts.append(a)
    blob = np.concatenate(parts)
    return [{"blob": blob} for _ in range(num_cores)]


def get_nc(**kw):
    key = ("nc", tuple(sorted(kw.items())))
    if key not in _CACHE:
        _CACHE[key] = build_nc(**kw)
    return _CACHE[key]


def kernel(**inputs) -> np.ndarray:
    nc = get_nc()
    maps = make_in_maps(inputs, 1)
    res = run_bass_kernel_spmd(nc, maps, [0])
    out = res.results[0]["out"].reshape(B, T, D)
    return out.astype(np.float32)


if __name__ == "__main__":
    import reference
    inputs = {k: np.asarray(v) for k, v in reference.setup_inputs().items()}
    got = kernel(**inputs)
    print("kernel ran; out shape", got.shape)



# revision 2
# speedup vs baseline: 3.6091x; 3.6091x over previous
"""DeepFusionCrossMamba Trainium2 kernel, v3.

Data-parallel over batch: 4 cores x 1 batch (the sharding hint).
Each core runs a single-batch program; the host test harness drives the
four cores with independently AOT-compiled single-device executables
dispatched from persistent threads (the shard_map multi-device path
costs ~4ms/call of dispatch; four independent dispatches overlap to
~1.6ms). Engine assignment uses measured HW rates (DVE 1.07/0.54
ns/elem f32/bf16, ACT 0.87, GPS 2.15, matmul ~0.84us per [128,512])
and the measured fact that GPSIMD serializes against DVE (shared SBUF
ports) while ACT/PE overlap DVE freely:

- DVE: scans (sp=4 packing), dBu multiplies (bf16 2x, broadcast-read u),
  gate mults, rmsnorm apply.
- GPS: hc = hsc*cm (bf16), u, residual adds, psum->sbuf bulk copies.
- PE:  all matmuls incl. depthwise conv as 4 accumulating diag matmuls
  (diagonalized conv + D_skip weights precomputed on host).
- ACT: exps (function-grouped to avoid 1.3us table reloads), softplus,
  silus, small psum->sbuf copies. No DMAs on the ACT queue.
- SP(sync): every dma_start + dma_start_transpose (feature<->time-major
  transposes moved off the PE).
"""

import numpy as np

import concourse.bass as bass
import concourse.bacc as bacc
import concourse.mybir as mybir
import concourse.tile as tile
from concourse.bass import IndirectOffsetOnAxis
from concourse.bass_utils import run_bass_kernel_spmd

F32 = mybir.dt.float32
BF16 = mybir.dt.bfloat16
I32 = mybir.dt.int32
AF = mybir.ActivationFunctionType
OP = mybir.AluOpType

B, T, D = 4, 1024, 256
NM, DI, DS, DCONV, DTR = 2, 512, 16, 4, 16
NDCH = D // 128    # 2 feature chunks of d_model
NICH = DI // 128   # 4 feature chunks of d_inner
NTB = T // 128     # 8 time blocks
SP = 4             # s-values packed per scan instruction
NG = DS // SP      # s-groups
EPS = 1e-8
NBPC = 1           # batches per core
N_CORES = 4

_CACHE = {}

PACK = [
    ("a_fm", [NBPC, D, T], "h"),
    ("v_fm", [NBPC, D, T], "h"),
    ("w_a2v", [D, D], "h"),
    ("b_a2v", [128, NDCH], "f"),
    ("w_v2a", [D, D], "h"),
    ("b_v2a", [128, NDCH], "f"),
    ("pk", [3, D, D], "h"),
    ("bn_s", [128, NDCH], "f"),
    ("bn_b", [128, NDCH], "f"),
    ("rmsw_bc", [NM, 128, D], "f"),
    ("w_in", [NM, 2, D, 2 * DI], "h"),
    ("cwd", [NM, 2, 128, NICH * DCONV * 128], "h"),  # diag(conv taps), row-major
    ("cb", [NM, 2, 128, NICH], "f"),
    ("w_xp", [NM, 2, DI, 80], "h"),
    ("w_dt", [NM, 2, DTR, DI], "h"),
    ("dtb", [NM, 2, 128, NICH], "f"),
    ("a_neg", [NM, 2, 128, NICH * DS], "f"),
    ("dskd", [NM, 2, 128, NICH * 128], "h"),         # diag(D_skip)
    ("w_out", [NM, 2, DI, D], "h"),
    ("g_bc", [128, D], "f"),
    ("be_bc", [128, D], "f"),
    ("id128", [128, 128], "f"),
    ("idx", [128, NTB], "i"),
]


def _pack_offsets():
    offs, off = {}, 0
    for name, shape, dt in PACK:
        n = 1
        for s in shape:
            n *= s
        if dt == "h":
            assert n % 2 == 0
            n //= 2
        offs[name] = (off, list(shape), dt)
        off += n
    return offs, off


OFFS, BLOB_N = _pack_offsets()


def _decl(nc, name, shape, dtype=F32, out=False):
    return nc.declare_dram_parameter(name, list(shape), dtype, isOutput=out)


def build_nc(nlayers=NM, nbatches=NBPC, probe=()):
    nc = bacc.Bacc(None, target_bir_lowering=False, debug=False)
    blob_d = _decl(nc, "blob", [BLOB_N])
    out_d = _decl(nc, "out", [NBPC * T, D], BF16, out=True)
    with tile.TileContext(nc) as tc:
        _body(nc, tc, nlayers, nbatches, blob_d, out_d, probe)
    nc.finalize()
    return nc


def _body(nc, tc, nlayers, nbatches, blob_d, out_d, probe=()):
    from contextlib import ExitStack
    ctx = ExitStack()
    with ctx:
        perm = ctx.enter_context(tc.tile_pool(name="perm", bufs=1))
        pwt = ctx.enter_context(tc.tile_pool(name="pwt", bufs=2))
        pwork = ctx.enter_context(tc.tile_pool(name="pwork", bufs=1))
        pscan = ctx.enter_context(tc.tile_pool(name="pscan", bufs=2))
        pbc = ctx.enter_context(tc.tile_pool(name="pbc", bufs=2))
        pdbu = ctx.enter_context(tc.tile_pool(name="pdbu", bufs=1))
        psmall = ctx.enter_context(tc.tile_pool(name="psmall", bufs=2))
        ptrans = ctx.enter_context(tc.tile_pool(name="ptrans", bufs=3,
                                                space="PSUM"))
        ppsy = ctx.enter_context(tc.tile_pool(name="ppsy", bufs=1,
                                              space="PSUM"))
        ppso = ctx.enter_context(tc.tile_pool(name="ppso", bufs=1,
                                              space="PSUM"))

        th_sl = [slice(0, 512), slice(512, 1024)]
        blob_ap = blob_d[:]

        def bsl(name, *pre, rows=None):
            off, shape, dt = OFFS[name]
            div = 2 if dt == "h" else 1
            for i, ix in enumerate(pre):
                stride = 1
                for s in shape[i + 1:]:
                    stride *= s
                off += ix * stride // div
            s = shape[len(pre):]
            assert len(s) == 2
            r0, r1 = (0, s[0]) if rows is None else rows
            off += r0 * s[1] // div
            n = (r1 - r0) * s[1] // div
            ap = blob_ap[off:off + n].rearrange("(a b) -> a b", a=r1 - r0)
            if dt == "h":
                ap = ap.bitcast(BF16)
            elif dt == "i":
                ap = ap.bitcast(I32)
            return ap

        def load(dram, shape, name, dtype=F32, pool=perm):
            if not isinstance(dram, bass.AP):
                dram = dram[:]
            t = pool.tile(shape, dtype, tag=name, name=name)
            nc.sync.dma_start(t[:], dram)
            return t

        # ---------------- shared persistent loads ----------------
        w_a2v = [load(bsl("w_a2v", rows=(c * 128, (c + 1) * 128)), [128, D],
                      f"w_a2v{c}", BF16) for c in range(NDCH)]
        w_v2a = [load(bsl("w_v2a", rows=(c * 128, (c + 1) * 128)), [128, D],
                      f"w_v2a{c}", BF16) for c in range(NDCH)]
        b_a2v = load(bsl("b_a2v"), [128, NDCH], "b_a2v")
        b_v2a = load(bsl("b_v2a"), [128, NDCH], "b_v2a")
        pk = [[load(bsl("pk", k, rows=(c * 128, (c + 1) * 128)), [128, D],
                    f"pk{k}{c}", BF16)
               for c in range(NDCH)] for k in range(3)]
        bn_s = load(bsl("bn_s"), [128, NDCH], "bn_s")
        bn_b = load(bsl("bn_b"), [128, NDCH], "bn_b")
        g_bc = load(bsl("g_bc"), [128, D], "g_bc")
        be_bc = load(bsl("be_bc"), [128, D], "be_bc")
        idx = load(bsl("idx"), [128, NTB], "idx", I32)
        id128 = load(bsl("id128"), [128, 128], "id128")
        id128b = perm.tile([128, 128], BF16, tag="id128b", name="id128b")
        nc.vector.tensor_copy(out=id128b[:], in_=id128[:])
        rmsw = [load(bsl("rmsw_bc", l), [128, D], f"rmsw{l}")
                for l in range(nlayers)]

        dAp = []
        for i in range(2):
            t = perm.tile([128, SP * T], BF16, tag=f"dA{i}", name=f"dA{i}")
            nc.vector.memset(t[:, 0:SP * T:T], 0.0)
            dAp.append(t)

        bounceY = [nc.dram_tensor(f"bounceY{i}", [T, D], BF16)
                   for i in range(2)]
        xdbl_dram = nc.dram_tensor("xdbl_dram", [32, T], BF16)

        def brd(src_rows_ap, n):
            # broadcast n DRAM rows of length T to 128 partitions
            return src_rows_ap.rearrange("a b -> (a b)").partition_broadcast(128)

        for bi in range(nbatches):
            # ================ preamble (feature-major) ================
            a_fm = [load(bsl("a_fm", bi, rows=(c * 128, (c + 1) * 128)),
                         [128, T], f"a_fm{c}", BF16, pool=pwork)
                    for c in range(NDCH)]
            # a_tm[j-block layout]: time-major audio via DMA transpose
            a_tm = perm.tile([128, NTB * D], BF16, tag="a_tm", name="a_tm")
            for j in range(NTB):
                for dc in range(NDCH):
                    nc.sync.dma_start_transpose(
                        a_tm[:, j * D + dc * 128: j * D + (dc + 1) * 128],
                        a_fm[dc][:, j * 128:(j + 1) * 128])
            g_a2v = [pwork.tile([128, T], BF16, tag=f"gate{c}",
                                name=f"g_a2v{c}") for c in range(NDCH)]
            for ec in range(NDCH):
                for th in range(2):
                    ps = ptrans.tile([128, 512], F32, tag="ps", name="ps")
                    for dc in range(NDCH):
                        nc.tensor.matmul(ps[:],
                                         w_a2v[dc][:, ec * 128:(ec + 1) * 128],
                                         a_fm[dc][:, th_sl[th]],
                                         start=(dc == 0), stop=(dc == NDCH - 1))
                    nc.scalar.activation(g_a2v[ec][:, th_sl[th]], ps[:],
                                         AF.Sigmoid, bias=b_a2v[:, ec:ec + 1])
            v_fm = [load(bsl("v_fm", bi, rows=(c * 128, (c + 1) * 128)),
                         [128, T], f"a_fm{c}", BF16, pool=pwork)
                    for c in range(NDCH)]
            v_pad = [pwork.tile([128, T + 3], BF16, tag=f"v_pad{c}",
                                name=f"v_pad{c}") for c in range(NDCH)]
            for c in range(NDCH):
                nc.vector.memset(v_pad[c][:, 0:1], 0.0)
                nc.vector.memset(v_pad[c][:, T + 1:T + 2], 0.0)
                nc.gpsimd.tensor_tensor(out=v_pad[c][:, 1:T + 1],
                                        in0=v_fm[c][:], in1=g_a2v[c][:],
                                        op=OP.mult)
            g_v2a = [pwork.tile([128, T], BF16, tag=f"gate{c+2}",
                                name=f"g_v2a{c}") for c in range(NDCH)]
            for ec in range(NDCH):
                for th in range(2):
                    ps = ptrans.tile([128, 512], F32, tag="ps", name="ps")
                    for dc in range(NDCH):
                        nc.tensor.matmul(ps[:],
                                         w_v2a[dc][:, ec * 128:(ec + 1) * 128],
                                         v_pad[dc][:, th * 512 + 1:
                                                    th * 512 + 513],
                                         start=(dc == 0), stop=(dc == NDCH - 1))
                    nc.scalar.activation(g_v2a[ec][:, th_sl[th]], ps[:],
                                         AF.Sigmoid, bias=b_v2a[:, ec:ec + 1])
            dlt = [pwork.tile([128, T], BF16, tag=f"xcs{c}", name=f"dlt{c}")
                   for c in range(NDCH)]
            for ec in range(NDCH):
                for th in range(2):
                    ps = ptrans.tile([128, 512], F32, tag="ps", name="ps")
                    first = True
                    for k in range(3):
                        for dc in range(NDCH):
                            nc.tensor.matmul(
                                ps[:], pk[k][dc][:, ec * 128:(ec + 1) * 128],
                                v_pad[dc][:, th * 512 + k: th * 512 + k + 512],
                                start=first, stop=(k == 2 and dc == NDCH - 1))
                            first = False
                    nc.scalar.activation(dlt[ec][:, th_sl[th]], ps[:],
                                         AF.Gelu, bias=bn_b[:, ec:ec + 1],
                                         scale=bn_s[:, ec:ec + 1])
            gdlt = [pwork.tile([128, T], BF16, tag=f"xcs{c+2}",
                               name=f"gdlt{c}") for c in range(NDCH)]
            for c in range(NDCH):
                nc.gpsimd.tensor_tensor(out=gdlt[c][:], in0=g_v2a[c][:],
                                        in1=dlt[c][:], op=OP.mult)
            # x0 (time-major f32) = a_tm + transpose(gdlt)
            gd_tm = pwork.tile([128, NTB * D], BF16, tag="h_tm",
                               name="gd_tm")
            for j in range(NTB):
                for dc in range(NDCH):
                    nc.sync.dma_start_transpose(
                        gd_tm[:, j * D + dc * 128: j * D + (dc + 1) * 128],
                        gdlt[dc][:, j * 128:(j + 1) * 128])
            x_tm = perm.tile([128, NTB * D], BF16, tag="x_tm0",
                             name="x_tm0")
            nc.vector.tensor_tensor(out=x_tm[:], in0=a_tm[:], in1=gd_tm[:],
                                    op=OP.add)

            # ================ mamba layers ================
            for l in range(nlayers):
                # rmsnorm over channel dim
                st = psmall.tile([128, NTB], F32, tag="st", name="st")
                sq = pwork.tile([128, D], F32, tag="sq", name="sq")
                for j in range(NTB):
                    nc.scalar.activation(sq[:], x_tm[:, j * D:(j + 1) * D],
                                         AF.Square, accum_out=st[:, j:j + 1])
                ms = psmall.tile([128, NTB], F32, tag="ms", name="ms")
                nc.vector.tensor_scalar(out=ms[:], in0=st[:],
                                        scalar1=1.0 / D, scalar2=1e-5,
                                        op0=OP.mult, op1=OP.add)
                msr = psmall.tile([128, NTB], F32, tag="msr", name="msr")
                nc.vector.reciprocal(out=msr[:], in_=ms[:])
                rstd = psmall.tile([128, NTB], F32, tag="rstd", name="rstd")
                nc.scalar.activation(rstd[:], msr[:], AF.Sqrt)
                h_tm = pwork.tile([128, NTB * D], BF16, tag="h_tm",
                                  name="h_tm")
                for j in range(NTB):
                    nc.vector.scalar_tensor_tensor(
                        out=h_tm[:, j * D:(j + 1) * D],
                        in0=x_tm[:, j * D:(j + 1) * D],
                        scalar=rstd[:, j:j + 1], in1=rmsw[l][:],
                        op0=OP.mult, op1=OP.mult)
                h_fm = [pwork.tile([128, T], BF16, tag=f"h_fm{c}",
                                   name=f"h_fm{c}") for c in range(NDCH)]
                for j in range(NTB):
                    for dc in range(NDCH):
                        nc.sync.dma_start_transpose(
                            h_fm[dc][:, j * 128:(j + 1) * 128],
                            h_tm[:, j * D + dc * 128: j * D + (dc + 1) * 128])
                h_rev_fm = [h_fm[c][:, ::-1] for c in range(NDCH)]

                ytm = [None, None]
                x_new = None
                for r in range(2):
                    hsrc = h_fm if r == 0 else h_rev_fm
                    w_in_l = [load(bsl("w_in", l, r,
                                       rows=(c * 128, (c + 1) * 128)),
                                   [128, 2 * DI], f"w_in{c}", BF16, pool=pwt)
                              for c in range(NDCH)]
                    cwd_t = load(bsl("cwd", l, r),
                                 [128, NICH * DCONV * 128], "cwd", BF16,
                                 pool=pwt)
                    cwd_l = [[cwd_t[:, (c * DCONV + k) * 128:
                                    (c * DCONV + k + 1) * 128]
                              for k in range(DCONV)] for c in range(NICH)]
                    cb_l = load(bsl("cb", l, r), [128, NICH], "cb", pool=pwt)
                    w_xp_l = [load(bsl("w_xp", l, r,
                                       rows=(c * 128, (c + 1) * 128)),
                                   [128, 80], f"w_xp{c}", BF16, pool=pwt)
                              for c in range(NICH)]
                    w_dt_l = load(bsl("w_dt", l, r), [DTR, DI], "w_dt",
                                  BF16, pool=pwt)
                    dtb_l = load(bsl("dtb", l, r), [128, NICH], "dtb",
                                 pool=pwt)
                    a_neg_l = load(bsl("a_neg", l, r), [128, NICH * DS],
                                   "a_neg", pool=pwt)
                    dskd_t = load(bsl("dskd", l, r), [128, NICH * 128],
                                  "dskd", BF16, pool=pwt)
                    dskd_l = [dskd_t[:, c * 128:(c + 1) * 128]
                              for c in range(NICH)]
                    w_out_l = [load(bsl("w_out", l, r,
                                        rows=(c * 128, (c + 1) * 128)),
                                    [128, D], f"w_out{c}", BF16, pool=pwt)
                               for c in range(NICH)]

                    # ---- in_proj -> xc (padded bf16), silu(z) ----
                    xc_pad = [pwork.tile([128, T + 3], BF16,
                                         tag=f"xc_pad{c}", name=f"xc_pad{c}")
                              for c in range(NICH)]
                    g_z = [pwork.tile([128, T], BF16, tag=f"gate{c}",
                                      name=f"g_z{c}") for c in range(NICH)]
                    for c in range(NICH):
                        nc.vector.memset(xc_pad[c][:, 0:3], 0.0)
                    for ec in range(2 * NICH):
                        for th in range(2):
                            ps = ptrans.tile([128, 512], F32, tag="ps",
                                             name="ps")
                            for dc in range(NDCH):
                                nc.tensor.matmul(
                                    ps[:],
                                    w_in_l[dc][:, ec * 128:(ec + 1) * 128],
                                    hsrc[dc][:, th_sl[th]],
                                    start=(dc == 0), stop=(dc == NDCH - 1))
                            if ec < NICH:
                                nc.scalar.copy(
                                    xc_pad[ec][:, 3 + th * 512:
                                               3 + th * 512 + 512], ps[:])
                            else:
                                nc.scalar.activation(
                                    g_z[ec - NICH][:, th_sl[th]], ps[:],
                                    AF.Silu)
                    # ---- depthwise conv: 4 accumulating diag matmuls ----
                    xc_s = [pwork.tile([128, T], BF16, tag=f"xcs{c}",
                                       name=f"xc_s{c}") for c in range(NICH)]
                    for c in range(NICH):
                        for th in range(2):
                            psc = ptrans.tile([128, 512], F32, tag="ps",
                                              name="ps")
                            for k in range(DCONV):
                                nc.tensor.matmul(
                                    psc[:], cwd_l[c][k],
                                    xc_pad[c][:, th * 512 + k:
                                              th * 512 + k + 512],
                                    start=(k == 0), stop=(k == DCONV - 1))
                            nc.scalar.activation(xc_s[c][:, th_sl[th]],
                                                 psc[:], AF.Silu,
                                                 bias=cb_l[:, c:c + 1])
                    # ---- x_proj -> xdbl rows [dt(16) | B(16) | C(16)] ----
                    xdbl = pwork.tile([DTR, T], BF16, tag="xdbl",
                                      name="xdbl")
                    xdbl_bf = pwork.tile([48, T], BF16, tag="xdbl_bf",
                                         name="xdbl_bf")
                    for th in range(2):
                        psx = ptrans.tile([80, 512], F32, tag="ps",
                                          name="ps")
                        for c in range(NICH):
                            nc.tensor.matmul(psx[:], w_xp_l[c][:],
                                             xc_s[c][:, th_sl[th]],
                                             start=(c == 0),
                                             stop=(c == NICH - 1))
                        nc.scalar.copy(xdbl[0:DTR, th_sl[th]], psx[0:DTR, :])
                        nc.scalar.copy(xdbl_bf[0:16, th_sl[th]],
                                       psx[32:48, :])
                        nc.scalar.copy(xdbl_bf[32:48, th_sl[th]],
                                       psx[64:80, :])
                        nc.sync.dma_start(xdbl_dram[0:16, th_sl[th]],
                                          xdbl_bf[0:16, th_sl[th]])
                        nc.sync.dma_start(xdbl_dram[16:32, th_sl[th]],
                                          xdbl_bf[32:48, th_sl[th]])

                    # ---- prep: delta (bf16), u (bf16), pipelined 2 ahead
                    # (delta/u rotate on c%2 tags; prep(c+2) is emitted
                    # after chunk c's exps so the ACT queue stays acyclic)
                    deltas, us = [None] * NICH, [None] * NICH

                    def prep(c):
                        psd = [ptrans.tile([128, 512], F32, tag="ps",
                                           name="ps") for _ in range(2)]
                        for th in range(2):
                            nc.tensor.matmul(psd[th][:],
                                             w_dt_l[:, c * 128:(c + 1) * 128],
                                             xdbl[0:DTR, th_sl[th]],
                                             start=True, stop=True)
                        esp = pwork.tile([128, T], BF16, tag="esp",
                                         name="esp")
                        for th in range(2):
                            nc.scalar.activation(esp[:, th_sl[th]],
                                                 psd[th][:], AF.Exp,
                                                 bias=dtb_l[:, c:c + 1])
                        delta = pwork.tile([128, T], BF16,
                                           tag=f"delta{c % 2}",
                                           name=f"delta{c % 2}")
                        nc.scalar.activation(delta[:], esp[:], AF.Ln,
                                             bias=1.0)
                        deltas[c] = delta
                        u = pwork.tile([128, T], BF16, tag=f"u{c % 2}",
                                       name=f"u{c % 2}")
                        nc.vector.tensor_tensor(out=u[:], in0=delta[:],
                                                in1=xc_s[c][:], op=OP.mult)
                        us[c] = u

                    prep(0)
                    prep(1)

                    # ---- scan section ----
                    y_g = [pwork.tile([128, T], BF16, tag=f"y_g{c}",
                                      name=f"y_g{c}") for c in range(NICH)]
                    for c in range(NICH):
                        psy = [ppsy.tile([128, 512], F32, tag=f"psy{th}",
                                         name=f"psy{th}") for th in range(2)]
                        for g in range(NG):
                            s0 = g * SP
                            dA = dAp[g % 2]
                            if "no_exp" not in probe:
                                for sv in range(SP):
                                    nc.scalar.activation(
                                        dA[:, sv * T + 1:(sv + 1) * T],
                                        deltas[c][:, 1:T], AF.Exp,
                                        scale=a_neg_l[:, c * DS + s0 + sv:
                                                      c * DS + s0 + sv + 1])
                            bm = pbc.tile([128, SP * T], BF16, tag="bm",
                                          name="bm")
                            nc.sync.dma_start(
                                bm[:], brd(xdbl_dram[s0:s0 + SP, :], SP))
                            cm = pbc.tile([128, SP * T], BF16, tag="cm",
                                          name="cm")
                            nc.sync.dma_start(
                                cm[:], brd(xdbl_dram[16 + s0:16 + s0 + SP, :],
                                           SP))
                            dBu = pdbu.tile([128, SP * T], BF16, tag="dBu",
                                            name="dBu")
                            if "no_dbu" not in probe:
                                nc.vector.tensor_tensor(
                                out=dBu[:].rearrange("p (a t) -> p a t",
                                                     a=SP),
                                in0=us[c][:].rearrange("p (a t) -> p a t",
                                                       a=1)
                                .broadcast_to([128, SP, T]),
                                    in1=bm[:].rearrange("p (a t) -> p a t",
                                                        a=SP),
                                    op=OP.mult)
                            hsc = pscan.tile([128, SP * T], BF16, tag="hsc",
                                             name="hsc")
                            if "scan_copy" in probe:
                                nc.vector.tensor_copy(out=hsc[:],
                                                      in_=dBu[:])
                            else:
                                nc.vector.tensor_tensor_scan(
                                    hsc[:], dA[:], dBu[:], 0.0,
                                    OP.mult, OP.add)
                            hc = pbc.tile([128, SP * T], BF16, tag="hc",
                                          name="hc")
                            heng = nc.gpsimd if "hc_gps" in probe \
                                else nc.vector
                            heng.tensor_tensor(out=hc[:], in0=hsc[:],
                                               in1=cm[:], op=OP.mult)
                            for sv in range(SP):
                                if "no_psy" in probe and not (g == 0 and sv == 0):
                                    continue
                                for th in range(2):
                                    nc.tensor.matmul(
                                        psy[th][:], id128b[:],
                                        hc[:, sv * T + th * 512:
                                           sv * T + th * 512 + 512],
                                        start=(g == 0 and sv == 0),
                                        stop=False)
                        for th in range(2):
                            nc.tensor.matmul(psy[th][:], dskd_l[c],
                                             xc_s[c][:, th_sl[th]],
                                             start=False, stop=True)
                        for th in range(2):
                            nc.vector.tensor_tensor(out=y_g[c][:, th_sl[th]],
                                                    in0=psy[th][:],
                                                    in1=g_z[c][:, th_sl[th]],
                                                    op=OP.mult)
                        if c + 2 < NICH:
                            prep(c + 2)

                    # ---- out_proj (time-major j-blocks); fwd residual
                    # add folds straight out of PSUM on DVE ----
                    if r == 0:
                        x_new = perm.tile([128, NTB * D], BF16,
                                          tag=f"x_tm{(l + 1) % 2}",
                                          name=f"x_tm{(l + 1) % 2}")
                    else:
                        ytm[1] = pwork.tile([128, NTB * D], BF16,
                                            tag="ytm1", name="ytm1")
                    for jh in range(2):
                        pso = ppso.tile([128, 4 * D], F32, tag="pso",
                                        name="pso")
                        for j4 in range(4):
                            j = jh * 4 + j4
                            for c in range(NICH):
                                nc.tensor.matmul(
                                    pso[:, j4 * D:(j4 + 1) * D],
                                    y_g[c][:, j * 128:(j + 1) * 128],
                                    w_out_l[c][:],
                                    start=(c == 0), stop=(c == NICH - 1))
                        if r == 0:
                            nc.vector.tensor_tensor(
                                out=x_new[:, jh * 4 * D:(jh + 1) * 4 * D],
                                in0=pso[:],
                                in1=x_tm[:, jh * 4 * D:(jh + 1) * 4 * D],
                                op=OP.add)
                        else:
                            nc.scalar.copy(
                                ytm[1][:, jh * 4 * D:(jh + 1) * 4 * D],
                                pso[:])

                # reverse bwd output via bounce + indirect row gather
                by = bounceY[l % 2]
                nc.sync.dma_start(
                    by[:].rearrange("(j p) d -> p j d", p=128),
                    ytm[1][:].rearrange("p (j d) -> p j d", j=NTB))
                artm = pwork.tile([128, NTB * D], BF16, tag="ytm1",
                                  name="artm")
                for j in range(NTB):
                    nc.gpsimd.indirect_dma_start(
                        out=artm[:, j * D:(j + 1) * D], out_offset=None,
                        in_=by[:],
                        in_offset=IndirectOffsetOnAxis(ap=idx[:, j:j + 1],
                                                       axis=0))
                nc.vector.tensor_tensor(out=x_new[:], in0=x_new[:],
                                        in1=artm[:], op=OP.add)
                x_tm = x_new

            # ================ final channel LayerNorm ================
            s_t = pwork.tile([128, NTB * D], BF16, tag="h_tm", name="s_t")
            nc.vector.tensor_tensor(out=s_t[:], in0=x_tm[:], in1=a_tm[:],
                                    op=OP.add)
            stm = psmall.tile([128, NTB], F32, tag="stm", name="stm")
            stv = psmall.tile([128, NTB], F32, tag="stv", name="stv")
            dump = pwork.tile([128, D], F32, tag="sq", name="dump")
            for j in range(NTB):
                nc.scalar.activation(dump[:], s_t[:, j * D:(j + 1) * D],
                                     AF.Copy, accum_out=stm[:, j:j + 1])
                nc.scalar.activation(dump[:], s_t[:, j * D:(j + 1) * D],
                                     AF.Square, accum_out=stv[:, j:j + 1])
            mu = psmall.tile([128, NTB], F32, tag="mu", name="mu")
            nc.vector.tensor_scalar(out=mu[:], in0=stm[:], scalar1=1.0 / D,
                                    scalar2=None, op0=OP.mult)
            var = psmall.tile([128, NTB], F32, tag="var", name="var")
            nc.vector.tensor_scalar(out=var[:], in0=stv[:], scalar1=1.0 / D,
                                    scalar2=None, op0=OP.mult)
            mu2 = psmall.tile([128, NTB], F32, tag="mu2", name="mu2")
            nc.vector.tensor_tensor(out=mu2[:], in0=mu[:], in1=mu[:],
                                    op=OP.mult)
            nc.vector.tensor_tensor(out=var[:], in0=var[:], in1=mu2[:],
                                    op=OP.subtract)
            ve = psmall.tile([128, NTB], F32, tag="ve", name="ve")
            nc.vector.tensor_scalar(out=ve[:], in0=var[:], scalar1=EPS,
                                    scalar2=None, op0=OP.add)
            vr = psmall.tile([128, NTB], F32, tag="vr", name="vr")
            nc.vector.reciprocal(out=vr[:], in_=ve[:])
            rstd2 = psmall.tile([128, NTB], F32, tag="rstd2", name="rstd2")
            nc.scalar.activation(rstd2[:], vr[:], AF.Sqrt)
            otm = pwork.tile([128, NTB * D], BF16, tag="gd_out", name="otm")
            for j in range(NTB):
                sl = slice(j * D, (j + 1) * D)
                nc.vector.tensor_scalar(out=otm[:, sl], in0=s_t[:, sl],
                                        scalar1=mu[:, j:j + 1],
                                        scalar2=rstd2[:, j:j + 1],
                                        op0=OP.subtract, op1=OP.mult)
            otm3 = otm[:].rearrange("p (j d) -> p j d", j=NTB)
            nc.vector.tensor_tensor(out=otm3, in0=otm3,
                                    in1=g_bc[:].rearrange(
                                        "p (a d) -> p a d", a=1)
                                    .broadcast_to([128, NTB, D]),
                                    op=OP.mult)
            nc.vector.tensor_tensor(out=otm3, in0=otm3,
                                    in1=be_bc[:].rearrange(
                                        "p (a d) -> p a d", a=1)
                                    .broadcast_to([128, NTB, D]),
                                    op=OP.add)
            nc.sync.dma_start(
                out_d[bi * T:(bi + 1) * T, :]
                .rearrange("(j p) d -> p j d", p=128),
                otm[:].rearrange("p (j d) -> p j d", j=NTB))


# ---------------- host side ----------------

def make_in_maps(inputs, num_cores=None):
    # one map per core; core i owns batch i (num_cores arg kept for
    # interface compatibility, the mesh is always N_CORES wide)
    inp = {k: np.asarray(v, dtype=np.float32) for k, v in inputs.items()}
    m = {}
    m["a_fm"] = np.ascontiguousarray(inp["audio"].transpose(0, 2, 1))
    m["v_fm"] = np.ascontiguousarray(inp["video"].transpose(0, 2, 1))
    m["w_a2v"] = np.ascontiguousarray(inp["gate_a2v_w"].T)
    m["b_a2v"] = np.ascontiguousarray(inp["gate_a2v_b"].reshape(NDCH, 128).T)
    m["w_v2a"] = np.ascontiguousarray(inp["gate_v2a_w"].T)
    m["b_v2a"] = np.ascontiguousarray(inp["gate_v2a_b"].reshape(NDCH, 128).T)
    m["pk"] = np.ascontiguousarray(
        np.stack([inp["proj_w"][:, :, k].T for k in range(3)]))
    m["bn_s"] = np.ascontiguousarray(
        (inp["bn_gamma"] / np.sqrt(1.0 + 1e-5)).reshape(NDCH, 128).T)
    m["bn_b"] = np.ascontiguousarray(inp["bn_beta"].reshape(NDCH, 128).T)
    m["rmsw_bc"] = np.ascontiguousarray(
        np.broadcast_to(inp["rms_w"][:, None, :], (NM, 128, D)))
    m["w_in"] = np.ascontiguousarray(inp["in_proj_w"].transpose(0, 1, 3, 2))
    # conv taps as diagonal matrices: cwd[l,r,c,k] = diag(conv_w[l,r,c*128:(c+1)*128,k])
    cw = inp["conv_w"]  # [NM,2,DI,DCONV]
    cwd = np.zeros((NM, 2, 128, NICH * DCONV * 128), np.float32)
    ii = np.arange(128)
    for l in range(NM):
        for r in range(2):
            for c in range(NICH):
                for k in range(DCONV):
                    cwd[l, r, ii, (c * DCONV + k) * 128 + ii] = \
                        cw[l, r, c * 128:(c + 1) * 128, k]
    m["cwd"] = cwd
    m["cb"] = np.ascontiguousarray(
        inp["conv_b"].reshape(NM, 2, NICH, 128).transpose(0, 1, 3, 2))
    w_xp_p = np.zeros((NM, 2, DI, 80), np.float32)
    for l in range(NM):
        for r in range(2):
            xp_t = inp["x_proj_w"][l, r].T  # [DI, 48]
            w_xp_p[l, r, :, 0:DTR] = xp_t[:, 0:DTR]
            w_xp_p[l, r, :, 32:32 + DS] = xp_t[:, DTR:DTR + DS]
            w_xp_p[l, r, :, 64:64 + DS] = xp_t[:, DTR + DS:DTR + 2 * DS]
    m["w_xp"] = w_xp_p
    m["w_dt"] = np.ascontiguousarray(inp["dt_w"].transpose(0, 1, 3, 2))
    m["dtb"] = np.ascontiguousarray(
        inp["dt_b"].reshape(NM, 2, NICH, 128).transpose(0, 1, 3, 2))
    m["a_neg"] = np.ascontiguousarray(
        (-np.exp(inp["A_log"])).reshape(NM, 2, NICH, 128, DS)
        .transpose(0, 1, 3, 2, 4).reshape(NM, 2, 128, NICH * DS))
    dsk = inp["D_skip"]  # [NM,2,DI]
    dskd = np.zeros((NM, 2, 128, NICH * 128), np.float32)
    for l in range(NM):
        for r in range(2):
            for c in range(NICH):
                dskd[l, r, ii, c * 128 + ii] = dsk[l, r, c * 128:(c + 1) * 128]
    m["dskd"] = dskd
    m["w_out"] = np.ascontiguousarray(inp["out_w"].transpose(0, 1, 3, 2))
    m["g_bc"] = np.ascontiguousarray(np.broadcast_to(inp["cln_gamma"],
                                                     (128, D)))
    m["be_bc"] = np.ascontiguousarray(np.broadcast_to(inp["cln_beta"],
                                                      (128, D)))
    m["id128"] = np.eye(128, dtype=np.float32)
    t_of = np.arange(T, dtype=np.int64).reshape(NTB, 128).T  # [128, NTB]
    m["idx"] = np.ascontiguousarray(((T - 1) - t_of).astype(np.int32))

    import ml_dtypes
    a_fm_all, v_fm_all = m["a_fm"], m["v_fm"]
    blobs = []
    for ci in range(N_CORES):
        m["a_fm"] = a_fm_all[ci:ci + 1]
        m["v_fm"] = v_fm_all[ci:ci + 1]
        parts = []
        for name, shape, dt in PACK:
            a = np.ascontiguousarray(m[name])
            assert a.shape == tuple(shape), (name, a.shape, shape)
            if dt == "h":
                a = a.astype(ml_dtypes.bfloat16).ravel().view(np.float32)
            elif dt == "i":
                a = a.view(np.float32).ravel()
            else:
                a = a.ravel()
            parts.append(a)
        blobs.append({"blob": np.concatenate(parts)})
    return blobs


def get_nc(**kw):
    key = ("nc", tuple(sorted(kw.items())))
    if key not in _CACHE:
        _CACHE[key] = build_nc(**kw)
    return _CACHE[key]


def kernel(**inputs) -> np.ndarray:
    nc = get_nc()
    maps = make_in_maps(inputs)
    res = run_bass_kernel_spmd(nc, maps, list(range(N_CORES)))
    out = np.stack([res.results[i]["out"].reshape(T, D)
                    for i in range(N_CORES)])
    return out.astype(np.float32)


if __name__ == "__main__":
    import reference
    inputs = {k: np.asarray(v) for k, v in reference.setup_inputs().items()}
    got = kernel(**inputs)
    print("kernel ran; out shape", got.shape)


# revision 6
# speedup vs baseline: 3.7348x; 1.0348x over previous
"""DeepFusionCrossMamba Trainium2 kernel, v3.

Data-parallel over batch: 4 cores x 1 batch (the sharding hint).
Each core runs a single-batch program; the host test harness drives the
four cores with independently AOT-compiled single-device executables
dispatched from persistent threads (the shard_map multi-device path
costs ~4ms/call of dispatch; four independent dispatches overlap to
~1.6ms). Engine assignment uses measured HW rates (DVE 1.07/0.54
ns/elem f32/bf16, ACT 0.87, GPS 2.15, matmul ~0.84us per [128,512])
and the measured fact that GPSIMD serializes against DVE (shared SBUF
ports) while ACT/PE overlap DVE freely:

- DVE: scans (sp=4 packing), dBu multiplies (bf16 2x, broadcast-read u),
  gate mults, rmsnorm apply.
- GPS: hc = hsc*cm (bf16), u, residual adds, psum->sbuf bulk copies.
- PE:  all matmuls incl. depthwise conv as 4 accumulating diag matmuls
  (diagonalized conv + D_skip weights precomputed on host).
- ACT: exps (function-grouped to avoid 1.3us table reloads), softplus,
  silus, small psum->sbuf copies. No DMAs on the ACT queue.
- SP(sync): every dma_start + dma_start_transpose (feature<->time-major
  transposes moved off the PE).
"""

import numpy as np

import concourse.bass as bass
import concourse.bacc as bacc
import concourse.mybir as mybir
import concourse.tile as tile
from concourse.bass import IndirectOffsetOnAxis
from concourse.bass_utils import run_bass_kernel_spmd

F32 = mybir.dt.float32
BF16 = mybir.dt.bfloat16
I32 = mybir.dt.int32
AF = mybir.ActivationFunctionType
OP = mybir.AluOpType

B, T, D = 4, 1024, 256
NM, DI, DS, DCONV, DTR = 2, 512, 16, 4, 16
NDCH = D // 128    # 2 feature chunks of d_model
NICH = DI // 128   # 4 feature chunks of d_inner
NTB = T // 128     # 8 time blocks
SP = 4             # s-values packed per scan instruction
NG = DS // SP      # s-groups
EPS = 1e-8
NBPC = 1           # batches per core
N_CORES = 4

_CACHE = {}

PACK = [
    ("a_fm", [NBPC, D, T], "h"),
    ("a_tmb", [NBPC, 128, NTB * D], "h"),
    ("v_fm", [NBPC, D, T], "h"),
    ("w_a2v", [D, D], "h"),
    ("b_a2v", [128, NDCH], "f"),
    ("w_v2a", [D, D], "h"),
    ("b_v2a", [128, NDCH], "f"),
    ("pk", [3, D, D], "h"),
    ("bn_s", [128, NDCH], "f"),
    ("bn_b", [128, NDCH], "f"),
    ("rmsw_bc", [NM, 128, D], "f"),
    ("w_in", [NM, 2, D, 2 * DI], "h"),
    ("cwd", [NM, 2, 128, NICH * DCONV * 128], "h"),  # diag(conv taps), row-major
    ("cb", [NM, 2, 128, NICH], "f"),
    ("w_xp", [NM, 2, DI, 80], "h"),
    ("w_dt", [NM, 2, DTR, DI], "h"),
    ("dtb", [NM, 2, 128, NICH], "f"),
    ("a_neg", [NM, 2, 128, NICH * DS], "f"),
    ("dskd", [NM, 2, 128, NICH * 128], "h"),         # diag(D_skip)
    ("w_out", [NM, 2, DI, D], "h"),
    ("g_bc", [128, D], "f"),
    ("be_bc", [128, D], "f"),
    ("id128", [128, 128], "f"),
    ("idx", [128, NTB], "i"),
]


def _pack_offsets():
    offs, off = {}, 0
    for name, shape, dt in PACK:
        n = 1
        for s in shape:
            n *= s
        if dt == "h":
            assert n % 2 == 0
            n //= 2
        offs[name] = (off, list(shape), dt)
        off += n
    return offs, off


OFFS, BLOB_N = _pack_offsets()


def _decl(nc, name, shape, dtype=F32, out=False):
    return nc.declare_dram_parameter(name, list(shape), dtype, isOutput=out)


def build_nc(nlayers=NM, nbatches=NBPC, probe=()):
    nc = bacc.Bacc(None, target_bir_lowering=False, debug=False)
    blob_d = _decl(nc, "blob", [BLOB_N])
    out_d = _decl(nc, "out", [NBPC * T, D], BF16, out=True)
    with tile.TileContext(nc) as tc:
        _body(nc, tc, nlayers, nbatches, blob_d, out_d, probe)
    nc.finalize()
    return nc


def _body(nc, tc, nlayers, nbatches, blob_d, out_d, probe=()):
    from contextlib import ExitStack
    ctx = ExitStack()
    with ctx:
        perm = ctx.enter_context(tc.tile_pool(name="perm", bufs=1))
        pwt = ctx.enter_context(tc.tile_pool(name="pwt", bufs=2))
        pwork = ctx.enter_context(tc.tile_pool(name="pwork", bufs=1))
        pscan = ctx.enter_context(tc.tile_pool(name="pscan", bufs=2))
        pbc = ctx.enter_context(tc.tile_pool(name="pbc", bufs=2))
        pdbu = ctx.enter_context(tc.tile_pool(name="pdbu", bufs=1))
        psmall = ctx.enter_context(tc.tile_pool(name="psmall", bufs=2))
        ptrans = ctx.enter_context(tc.tile_pool(name="ptrans", bufs=2,
                                                space="PSUM"))
        ppsy = ctx.enter_context(tc.tile_pool(name="ppsy", bufs=1,
                                              space="PSUM"))
        ppso = ctx.enter_context(tc.tile_pool(name="ppso", bufs=1,
                                              space="PSUM"))
        ppst = ctx.enter_context(tc.tile_pool(name="ppst", bufs=2,
                                              space="PSUM"))

        th_sl = [slice(0, 512), slice(512, 1024)]
        blob_ap = blob_d[:]

        def bsl(name, *pre, rows=None):
            off, shape, dt = OFFS[name]
            div = 2 if dt == "h" else 1
            for i, ix in enumerate(pre):
                stride = 1
                for s in shape[i + 1:]:
                    stride *= s
                off += ix * stride // div
            s = shape[len(pre):]
            assert len(s) == 2
            r0, r1 = (0, s[0]) if rows is None else rows
            off += r0 * s[1] // div
            n = (r1 - r0) * s[1] // div
            ap = blob_ap[off:off + n].rearrange("(a b) -> a b", a=r1 - r0)
            if dt == "h":
                ap = ap.bitcast(BF16)
            elif dt == "i":
                ap = ap.bitcast(I32)
            return ap

        def load(dram, shape, name, dtype=F32, pool=perm):
            if not isinstance(dram, bass.AP):
                dram = dram[:]
            t = pool.tile(shape, dtype, tag=name, name=name)
            nc.sync.dma_start(t[:], dram)
            return t

        # ---------------- shared persistent loads ----------------
        w_a2v = [load(bsl("w_a2v", rows=(c * 128, (c + 1) * 128)), [128, D],
                      f"w_a2v{c}", BF16) for c in range(NDCH)]
        w_v2a = [load(bsl("w_v2a", rows=(c * 128, (c + 1) * 128)), [128, D],
                      f"w_v2a{c}", BF16) for c in range(NDCH)]
        b_a2v = load(bsl("b_a2v"), [128, NDCH], "b_a2v")
        b_v2a = load(bsl("b_v2a"), [128, NDCH], "b_v2a")
        pk = [[load(bsl("pk", k, rows=(c * 128, (c + 1) * 128)), [128, D],
                    f"pk{k}{c}", BF16)
               for c in range(NDCH)] for k in range(3)]
        bn_s = load(bsl("bn_s"), [128, NDCH], "bn_s")
        bn_b = load(bsl("bn_b"), [128, NDCH], "bn_b")
        g_bc = load(bsl("g_bc"), [128, D], "g_bc")
        be_bc = load(bsl("be_bc"), [128, D], "be_bc")
        idx = load(bsl("idx"), [128, NTB], "idx", I32)
        id128 = load(bsl("id128"), [128, 128], "id128")
        id128b = perm.tile([128, 128], BF16, tag="id128b", name="id128b")
        nc.vector.tensor_copy(out=id128b[:], in_=id128[:])
        rmsw = [load(bsl("rmsw_bc", l), [128, D], f"rmsw{l}")
                for l in range(nlayers)]

        dAp = []
        for i in range(2):
            t = perm.tile([128, SP * T], BF16, tag=f"dA{i}", name=f"dA{i}")
            nc.vector.memset(t[:, 0:SP * T:T], 0.0)
            dAp.append(t)

        bounceY = [nc.dram_tensor(f"bounceY{i}", [T, D], BF16)
                   for i in range(2)]
        xdbl_dram = nc.dram_tensor("xdbl_dram", [32, T], BF16)

        def brd(src_rows_ap, n):
            # broadcast n DRAM rows of length T to 128 partitions
            return src_rows_ap.rearrange("a b -> (a b)").partition_broadcast(128)

        for bi in range(nbatches):
            # ================ preamble (feature-major) ================
            a_fm = [load(bsl("a_fm", bi, rows=(c * 128, (c + 1) * 128)),
                         [128, T], f"a_fm{c}", BF16, pool=pwork)
                    for c in range(NDCH)]
            # a_tm[j-block layout]: time-major audio direct from blob
            a_tm = load(bsl("a_tmb", bi), [128, NTB * D], "a_tm", BF16)
            g_a2v = [pwork.tile([128, T], BF16, tag=f"gate{c}",
                                name=f"g_a2v{c}") for c in range(NDCH)]
            for ec in range(NDCH):
                for th in range(2):
                    ps = ptrans.tile([128, 512], F32, tag="ps", name="ps")
                    for dc in range(NDCH):
                        nc.tensor.matmul(ps[:],
                                         w_a2v[dc][:, ec * 128:(ec + 1) * 128],
                                         a_fm[dc][:, th_sl[th]],
                                         start=(dc == 0), stop=(dc == NDCH - 1))
                    nc.scalar.activation(g_a2v[ec][:, th_sl[th]], ps[:],
                                         AF.Sigmoid, bias=b_a2v[:, ec:ec + 1])
            v_fm = [load(bsl("v_fm", bi, rows=(c * 128, (c + 1) * 128)),
                         [128, T], f"a_fm{c}", BF16, pool=pwork)
                    for c in range(NDCH)]
            v_pad = [pwork.tile([128, T + 3], BF16, tag=f"v_pad{c}",
                                name=f"v_pad{c}") for c in range(NDCH)]
            for c in range(NDCH):
                nc.vector.memset(v_pad[c][:, 0:1], 0.0)
                nc.vector.memset(v_pad[c][:, T + 1:T + 2], 0.0)
                nc.gpsimd.tensor_tensor(out=v_pad[c][:, 1:T + 1],
                                        in0=v_fm[c][:], in1=g_a2v[c][:],
                                        op=OP.mult)
            g_v2a = [pwork.tile([128, T], BF16, tag=f"gate{c+2}",
                                name=f"g_v2a{c}") for c in range(NDCH)]
            for ec in range(NDCH):
                for th in range(2):
                    ps = ptrans.tile([128, 512], F32, tag="ps", name="ps")
                    for dc in range(NDCH):
                        nc.tensor.matmul(ps[:],
                                         w_v2a[dc][:, ec * 128:(ec + 1) * 128],
                                         v_pad[dc][:, th * 512 + 1:
                                                    th * 512 + 513],
                                         start=(dc == 0), stop=(dc == NDCH - 1))
                    nc.scalar.activation(g_v2a[ec][:, th_sl[th]], ps[:],
                                         AF.Sigmoid, bias=b_v2a[:, ec:ec + 1])
            dlt = [pwork.tile([128, T], BF16, tag=f"xcs{c}", name=f"dlt{c}")
                   for c in range(NDCH)]
            for ec in range(NDCH):
                for th in range(2):
                    ps = ptrans.tile([128, 512], F32, tag="ps", name="ps")
                    first = True
                    for k in range(3):
                        for dc in range(NDCH):
                            nc.tensor.matmul(
                                ps[:], pk[k][dc][:, ec * 128:(ec + 1) * 128],
                                v_pad[dc][:, th * 512 + k: th * 512 + k + 512],
                                start=first, stop=(k == 2 and dc == NDCH - 1))
                            first = False
                    nc.scalar.activation(dlt[ec][:, th_sl[th]], ps[:],
                                         AF.Gelu, bias=bn_b[:, ec:ec + 1],
                                         scale=bn_s[:, ec:ec + 1])
            gdlt = [pwork.tile([128, T], BF16, tag=f"xcs{c+2}",
                               name=f"gdlt{c}") for c in range(NDCH)]
            for c in range(NDCH):
                nc.gpsimd.tensor_tensor(out=gdlt[c][:], in0=g_v2a[c][:],
                                        in1=dlt[c][:], op=OP.mult)
            # x0 (time-major f32) = a_tm + transpose(gdlt)
            gd_tm = pwork.tile([128, NTB * D], BF16, tag="h_tm",
                               name="gd_tm")
            for j in range(NTB):
                for dc in range(NDCH):
                    psT = ppst.tile([128, 128], BF16, tag="pst",
                                    name="pst")
                    nc.tensor.transpose(psT[:],
                                        gdlt[dc][:, j * 128:(j + 1) * 128],
                                        id128b[:])
                    nc.scalar.copy(
                        gd_tm[:, j * D + dc * 128: j * D + (dc + 1) * 128],
                        psT[:])
            x_tm = perm.tile([128, NTB * D], BF16, tag="x_tm0",
                             name="x_tm0")
            nc.vector.tensor_tensor(out=x_tm[:], in0=a_tm[:], in1=gd_tm[:],
                                    op=OP.add)

            # ================ mamba layers ================
            for l in range(nlayers):
                # rmsnorm over channel dim
                st = psmall.tile([128, NTB], F32, tag="st", name="st")
                sq = pwork.tile([128, D], F32, tag="sq", name="sq")
                for j in range(NTB):
                    nc.scalar.activation(sq[:], x_tm[:, j * D:(j + 1) * D],
                                         AF.Square, accum_out=st[:, j:j + 1])
                ms = psmall.tile([128, NTB], F32, tag="ms", name="ms")
                nc.vector.tensor_scalar(out=ms[:], in0=st[:],
                                        scalar1=1.0 / D, scalar2=1e-5,
                                        op0=OP.mult, op1=OP.add)
                msr = psmall.tile([128, NTB], F32, tag="msr", name="msr")
                nc.vector.reciprocal(out=msr[:], in_=ms[:])
                rstd = psmall.tile([128, NTB], F32, tag="rstd", name="rstd")
                nc.scalar.activation(rstd[:], msr[:], AF.Sqrt)
                h_tm = pwork.tile([128, NTB * D], BF16, tag="h_tm",
                                  name="h_tm")
                for j in range(NTB):
                    nc.vector.scalar_tensor_tensor(
                        out=h_tm[:, j * D:(j + 1) * D],
                        in0=x_tm[:, j * D:(j + 1) * D],
                        scalar=rstd[:, j:j + 1], in1=rmsw[l][:],
                        op0=OP.mult, op1=OP.mult)
                h_fm = [pwork.tile([128, T], BF16, tag=f"h_fm{c}",
                                   name=f"h_fm{c}") for c in range(NDCH)]
                for j in range(NTB):
                    for dc in range(NDCH):
                        psT = ppst.tile([128, 128], BF16, tag="pst",
                                        name="pst")
                        nc.tensor.transpose(
                            psT[:],
                            h_tm[:, j * D + dc * 128: j * D + (dc + 1) * 128],
                            id128b[:])
                        nc.scalar.copy(h_fm[dc][:, j * 128:(j + 1) * 128],
                                       psT[:])
                h_rev_fm = [h_fm[c][:, ::-1] for c in range(NDCH)]

                ytm = [None, None]
                x_new = None
                for r in range(2):
                    hsrc = h_fm if r == 0 else h_rev_fm
                    w_in_l = [load(bsl("w_in", l, r,
                                       rows=(c * 128, (c + 1) * 128)),
                                   [128, 2 * DI], f"w_in{c}", BF16, pool=pwt)
                              for c in range(NDCH)]
                    cwd_t = load(bsl("cwd", l, r),
                                 [128, NICH * DCONV * 128], "cwd", BF16,
                                 pool=pwt)
                    cwd_l = [[cwd_t[:, (c * DCONV + k) * 128:
                                    (c * DCONV + k + 1) * 128]
                              for k in range(DCONV)] for c in range(NICH)]
                    cb_l = load(bsl("cb", l, r), [128, NICH], "cb", pool=pwt)
                    w_xp_l = [load(bsl("w_xp", l, r,
                                       rows=(c * 128, (c + 1) * 128)),
                                   [128, 80], f"w_xp{c}", BF16, pool=pwt)
                              for c in range(NICH)]
                    w_dt_l = load(bsl("w_dt", l, r), [DTR, DI], "w_dt",
                                  BF16, pool=pwt)
                    dtb_l = load(bsl("dtb", l, r), [128, NICH], "dtb",
                                 pool=pwt)
                    a_neg_l = load(bsl("a_neg", l, r), [128, NICH * DS],
                                   "a_neg", pool=pwt)
                    dskd_t = load(bsl("dskd", l, r), [128, NICH * 128],
                                  "dskd", BF16, pool=pwt)
                    dskd_l = [dskd_t[:, c * 128:(c + 1) * 128]
                              for c in range(NICH)]
                    w_out_l = [load(bsl("w_out", l, r,
                                        rows=(c * 128, (c + 1) * 128)),
                                    [128, D], f"w_out{c}", BF16, pool=pwt)
                               for c in range(NICH)]

                    # ---- in_proj -> xc (padded bf16), silu(z) ----
                    xc_pad = [pwork.tile([128, T + 3], BF16,
                                         tag=f"xc_pad{c}", name=f"xc_pad{c}")
                              for c in range(NICH)]
                    g_z = [pwork.tile([128, T], BF16, tag=f"gate{c}",
                                      name=f"g_z{c}") for c in range(NICH)]
                    for c in range(NICH):
                        nc.vector.memset(xc_pad[c][:, 0:3], 0.0)
                    for ec in range(2 * NICH):
                        for th in range(2):
                            ps = ptrans.tile([128, 512], F32, tag="ps",
                                             name="ps")
                            for dc in range(NDCH):
                                nc.tensor.matmul(
                                    ps[:],
                                    w_in_l[dc][:, ec * 128:(ec + 1) * 128],
                                    hsrc[dc][:, th_sl[th]],
                                    start=(dc == 0), stop=(dc == NDCH - 1))
                            if ec < NICH:
                                nc.scalar.copy(
                                    xc_pad[ec][:, 3 + th * 512:
                                               3 + th * 512 + 512], ps[:])
                            else:
                                nc.scalar.activation(
                                    g_z[ec - NICH][:, th_sl[th]], ps[:],
                                    AF.Silu)
                    # ---- depthwise conv: 4 accumulating diag matmuls ----
                    xc_s = [pwork.tile([128, T], BF16, tag=f"xcs{c}",
                                       name=f"xc_s{c}") for c in range(NICH)]
                    for c in range(NICH):
                        for th in range(2):
                            psc = ptrans.tile([128, 512], F32, tag="ps",
                                              name="ps")
                            for k in range(DCONV):
                                nc.tensor.matmul(
                                    psc[:], cwd_l[c][k],
                                    xc_pad[c][:, th * 512 + k:
                                              th * 512 + k + 512],
                                    start=(k == 0), stop=(k == DCONV - 1))
                            nc.scalar.activation(xc_s[c][:, th_sl[th]],
                                                 psc[:], AF.Silu,
                                                 bias=cb_l[:, c:c + 1])
                    # ---- x_proj -> xdbl rows [dt(16) | B(16) | C(16)] ----
                    xdbl = pwork.tile([DTR, T], BF16, tag="xdbl",
                                      name="xdbl")
                    xdbl_bf = pwork.tile([48, T], BF16, tag="xdbl_bf",
                                         name="xdbl_bf")
                    for th in range(2):
                        psx = ptrans.tile([80, 512], F32, tag="ps",
                                          name="ps")
                        for c in range(NICH):
                            nc.tensor.matmul(psx[:], w_xp_l[c][:],
                                             xc_s[c][:, th_sl[th]],
                                             start=(c == 0),
                                             stop=(c == NICH - 1))
                        nc.scalar.copy(xdbl[0:DTR, th_sl[th]], psx[0:DTR, :])
                        nc.scalar.copy(xdbl_bf[0:16, th_sl[th]],
                                       psx[32:48, :])
                        nc.scalar.copy(xdbl_bf[32:48, th_sl[th]],
                                       psx[64:80, :])
                        nc.sync.dma_start(xdbl_dram[0:16, th_sl[th]],
                                          xdbl_bf[0:16, th_sl[th]])
                        nc.sync.dma_start(xdbl_dram[16:32, th_sl[th]],
                                          xdbl_bf[32:48, th_sl[th]])

                    # ---- prep: delta (bf16), u (bf16), pipelined 2 ahead
                    # (delta/u rotate on c%2 tags; prep(c+2) is emitted
                    # after chunk c's exps so the ACT queue stays acyclic)
                    deltas, us = [None] * NICH, [None] * NICH

                    def prep(c):
                        psd = [ptrans.tile([128, 512], F32, tag="ps",
                                           name="ps") for _ in range(2)]
                        for th in range(2):
                            nc.tensor.matmul(psd[th][:],
                                             w_dt_l[:, c * 128:(c + 1) * 128],
                                             xdbl[0:DTR, th_sl[th]],
                                             start=True, stop=True)
                        esp = pwork.tile([128, T], BF16, tag="esp",
                                         name="esp")
                        for th in range(2):
                            nc.scalar.activation(esp[:, th_sl[th]],
                                                 psd[th][:], AF.Exp,
                                                 bias=dtb_l[:, c:c + 1])
                        delta = pwork.tile([128, T], BF16,
                                           tag=f"delta{c % 2}",
                                           name=f"delta{c % 2}")
                        nc.scalar.activation(delta[:], esp[:], AF.Ln,
                                             bias=1.0)
                        deltas[c] = delta
                        u = pwork.tile([128, T], BF16, tag=f"u{c % 2}",
                                       name=f"u{c % 2}")
                        nc.vector.tensor_tensor(out=u[:], in0=delta[:],
                                                in1=xc_s[c][:], op=OP.mult)
                        us[c] = u

                    prep(0)
                    prep(1)

                    # ---- scan section ----
                    y_g = [pwork.tile([128, T], BF16, tag=f"y_g{c}",
                                      name=f"y_g{c}") for c in range(NICH)]
                    for c in range(NICH):
                        psy = [ppsy.tile([128, 512], F32, tag=f"psy{th}",
                                         name=f"psy{th}") for th in range(2)]
                        for g in range(NG):
                            s0 = g * SP
                            dA = dAp[g % 2]
                            if "no_exp" not in probe:
                                for sv in range(SP):
                                    nc.scalar.activation(
                                        dA[:, sv * T + 1:(sv + 1) * T],
                                        deltas[c][:, 1:T], AF.Exp,
                                        scale=a_neg_l[:, c * DS + s0 + sv:
                                                      c * DS + s0 + sv + 1])
                            bm = pbc.tile([128, SP * T], BF16, tag="bm",
                                          name="bm")
                            nc.sync.dma_start(
                                bm[:], brd(xdbl_dram[s0:s0 + SP, :], SP))
                            cm = pbc.tile([128, SP * T], BF16, tag="cm",
                                          name="cm")
                            nc.sync.dma_start(
                                cm[:], brd(xdbl_dram[16 + s0:16 + s0 + SP, :],
                                           SP))
                            dBu = pdbu.tile([128, SP * T], BF16, tag="dBu",
                                            name="dBu")
                            if "no_dbu" not in probe:
                                nc.vector.tensor_tensor(
                                out=dBu[:].rearrange("p (a t) -> p a t",
                                                     a=SP),
                                in0=us[c][:].rearrange("p (a t) -> p a t",
                                                       a=1)
                                .broadcast_to([128, SP, T]),
                                    in1=bm[:].rearrange("p (a t) -> p a t",
                                                        a=SP),
                                    op=OP.mult)
                            hsc = pscan.tile([128, SP * T], BF16, tag="hsc",
                                             name="hsc")
                            if "scan_copy" in probe:
                                nc.vector.tensor_copy(out=hsc[:],
                                                      in_=dBu[:])
                            else:
                                nc.vector.tensor_tensor_scan(
                                    hsc[:], dA[:], dBu[:], 0.0,
                                    OP.mult, OP.add)
                            hc = pbc.tile([128, SP * T], BF16, tag="hc",
                                          name="hc")
                            heng = nc.gpsimd if "hc_gps" in probe \
                                else nc.vector
                            heng.tensor_tensor(out=hc[:], in0=hsc[:],
                                               in1=cm[:], op=OP.mult)
                            for sv in range(SP):
                                if "no_psy" in probe and not (g == 0 and sv == 0):
                                    continue
                                for th in range(2):
                                    nc.tensor.matmul(
                                        psy[th][:], id128b[:],
                                        hc[:, sv * T + th * 512:
                                           sv * T + th * 512 + 512],
                                        start=(g == 0 and sv == 0),
                                        stop=False)
                        for th in range(2):
                            nc.tensor.matmul(psy[th][:], dskd_l[c],
                                             xc_s[c][:, th_sl[th]],
                                             start=False, stop=True)
                        for th in range(2):
                            nc.vector.tensor_tensor(out=y_g[c][:, th_sl[th]],
                                                    in0=psy[th][:],
                                                    in1=g_z[c][:, th_sl[th]],
                                                    op=OP.mult)
                        if c + 2 < NICH:
                            prep(c + 2)

                    # ---- out_proj (time-major j-blocks); fwd residual
                    # add folds straight out of PSUM on DVE ----
                    if r == 0:
                        x_new = perm.tile([128, NTB * D], BF16,
                                          tag=f"x_tm{(l + 1) % 2}",
                                          name=f"x_tm{(l + 1) % 2}")
                    else:
                        ytm[1] = pwork.tile([128, NTB * D], BF16,
                                            tag="ytm1", name="ytm1")
                    for jh in range(2):
                        pso = ppso.tile([128, 4 * D], F32, tag="pso",
                                        name="pso")
                        for j4 in range(4):
                            j = jh * 4 + j4
                            for c in range(NICH):
                                nc.tensor.matmul(
                                    pso[:, j4 * D:(j4 + 1) * D],
                                    y_g[c][:, j * 128:(j + 1) * 128],
                                    w_out_l[c][:],
                                    start=(c == 0), stop=(c == NICH - 1))
                        if r == 0:
                            nc.vector.tensor_tensor(
                                out=x_new[:, jh * 4 * D:(jh + 1) * 4 * D],
                                in0=pso[:],
                                in1=x_tm[:, jh * 4 * D:(jh + 1) * 4 * D],
                                op=OP.add)
                        else:
                            nc.scalar.copy(
                                ytm[1][:, jh * 4 * D:(jh + 1) * 4 * D],
                                pso[:])

                # reverse bwd output via bounce + indirect row gather
                by = bounceY[l % 2]
                nc.sync.dma_start(
                    by[:].rearrange("(j p) d -> p j d", p=128),
                    ytm[1][:].rearrange("p (j d) -> p j d", j=NTB))
                artm = pwork.tile([128, NTB * D], BF16, tag="ytm1",
                                  name="artm")
                for j in range(NTB):
                    nc.gpsimd.indirect_dma_start(
                        out=artm[:, j * D:(j + 1) * D], out_offset=None,
                        in_=by[:],
                        in_offset=IndirectOffsetOnAxis(ap=idx[:, j:j + 1],
                                                       axis=0))
                nc.vector.tensor_tensor(out=x_new[:], in0=x_new[:],
                                        in1=artm[:], op=OP.add)
                x_tm = x_new

            # ================ final channel LayerNorm ================
            s_t = pwork.tile([128, NTB * D], BF16, tag="h_tm", name="s_t")
            nc.vector.tensor_tensor(out=s_t[:], in0=x_tm[:], in1=a_tm[:],
                                    op=OP.add)
            stm = psmall.tile([128, NTB], F32, tag="stm", name="stm")
            stv = psmall.tile([128, NTB], F32, tag="stv", name="stv")
            dump = pwork.tile([128, D], F32, tag="sq", name="dump")
            for j in range(NTB):
                nc.scalar.activation(dump[:], s_t[:, j * D:(j + 1) * D],
                                     AF.Copy, accum_out=stm[:, j:j + 1])
                nc.scalar.activation(dump[:], s_t[:, j * D:(j + 1) * D],
                                     AF.Square, accum_out=stv[:, j:j + 1])
            mu = psmall.tile([128, NTB], F32, tag="mu", name="mu")
            nc.vector.tensor_scalar(out=mu[:], in0=stm[:], scalar1=1.0 / D,
                                    scalar2=None, op0=OP.mult)
            var = psmall.tile([128, NTB], F32, tag="var", name="var")
            nc.vector.tensor_scalar(out=var[:], in0=stv[:], scalar1=1.0 / D,
                                    scalar2=None, op0=OP.mult)
            mu2 = psmall.tile([128, NTB], F32, tag="mu2", name="mu2")
            nc.vector.tensor_tensor(out=mu2[:], in0=mu[:], in1=mu[:],
                                    op=OP.mult)
            nc.vector.tensor_tensor(out=var[:], in0=var[:], in1=mu2[:],
                                    op=OP.subtract)
            ve = psmall.tile([128, NTB], F32, tag="ve", name="ve")
            nc.vector.tensor_scalar(out=ve[:], in0=var[:], scalar1=EPS,
                                    scalar2=None, op0=OP.add)
            vr = psmall.tile([128, NTB], F32, tag="vr", name="vr")
            nc.vector.reciprocal(out=vr[:], in_=ve[:])
            rstd2 = psmall.tile([128, NTB], F32, tag="rstd2", name="rstd2")
            nc.scalar.activation(rstd2[:], vr[:], AF.Sqrt)
            otm = pwork.tile([128, NTB * D], BF16, tag="gd_out", name="otm")
            for j in range(NTB):
                sl = slice(j * D, (j + 1) * D)
                nc.vector.tensor_scalar(out=otm[:, sl], in0=s_t[:, sl],
                                        scalar1=mu[:, j:j + 1],
                                        scalar2=rstd2[:, j:j + 1],
                                        op0=OP.subtract, op1=OP.mult)
            otm3 = otm[:].rearrange("p (j d) -> p j d", j=NTB)
            nc.vector.tensor_tensor(out=otm3, in0=otm3,
                                    in1=g_bc[:].rearrange(
                                        "p (a d) -> p a d", a=1)
                                    .broadcast_to([128, NTB, D]),
                                    op=OP.mult)
            nc.vector.tensor_tensor(out=otm3, in0=otm3,
                                    in1=be_bc[:].rearrange(
                                        "p (a d) -> p a d", a=1)
                                    .broadcast_to([128, NTB, D]),
                                    op=OP.add)
            nc.sync.dma_start(
                out_d[bi * T:(bi + 1) * T, :]
                .rearrange("(j p) d -> p j d", p=128),
                otm[:].rearrange("p (j d) -> p j d", j=NTB))


# ---------------- host side ----------------

def make_in_maps(inputs, num_cores=None):
    # one map per core; core i owns batch i (num_cores arg kept for
    # interface compatibility, the mesh is always N_CORES wide)
    inp = {k: np.asarray(v, dtype=np.float32) for k, v in inputs.items()}
    m = {}
    m["a_fm"] = np.ascontiguousarray(inp["audio"].transpose(0, 2, 1))
    # time-major j-block layout: a_tmb[b, p, j*D + d] = audio[b, j*128+p, d]
    m["a_tmb"] = np.ascontiguousarray(
        inp["audio"].reshape(B, NTB, 128, D).transpose(0, 2, 1, 3)
        .reshape(B, 128, NTB * D))
    m["v_fm"] = np.ascontiguousarray(inp["video"].transpose(0, 2, 1))
    m["w_a2v"] = np.ascontiguousarray(inp["gate_a2v_w"].T)
    m["b_a2v"] = np.ascontiguousarray(inp["gate_a2v_b"].reshape(NDCH, 128).T)
    m["w_v2a"] = np.ascontiguousarray(inp["gate_v2a_w"].T)
    m["b_v2a"] = np.ascontiguousarray(inp["gate_v2a_b"].reshape(NDCH, 128).T)
    m["pk"] = np.ascontiguousarray(
        np.stack([inp["proj_w"][:, :, k].T for k in range(3)]))
    m["bn_s"] = np.ascontiguousarray(
        (inp["bn_gamma"] / np.sqrt(1.0 + 1e-5)).reshape(NDCH, 128).T)
    m["bn_b"] = np.ascontiguousarray(inp["bn_beta"].reshape(NDCH, 128).T)
    m["rmsw_bc"] = np.ascontiguousarray(
        np.broadcast_to(inp["rms_w"][:, None, :], (NM, 128, D)))
    m["w_in"] = np.ascontiguousarray(inp["in_proj_w"].transpose(0, 1, 3, 2))
    # conv taps as diagonal matrices: cwd[l,r,c,k] = diag(conv_w[l,r,c*128:(c+1)*128,k])
    cw = inp["conv_w"]  # [NM,2,DI,DCONV]
    cwd = np.zeros((NM, 2, 128, NICH * DCONV * 128), np.float32)
    ii = np.arange(128)
    for l in range(NM):
        for r in range(2):
            for c in range(NICH):
                for k in range(DCONV):
                    cwd[l, r, ii, (c * DCONV + k) * 128 + ii] = \
                        cw[l, r, c * 128:(c + 1) * 128, k]
    m["cwd"] = cwd
    m["cb"] = np.ascontiguousarray(
        inp["conv_b"].reshape(NM, 2, NICH, 128).transpose(0, 1, 3, 2))
    w_xp_p = np.zeros((NM, 2, DI, 80), np.float32)
    for l in range(NM):
        for r in range(2):
            xp_t = inp["x_proj_w"][l, r].T  # [DI, 48]
            w_xp_p[l, r, :, 0:DTR] = xp_t[:, 0:DTR]
            w_xp_p[l, r, :, 32:32 + DS] = xp_t[:, DTR:DTR + DS]
            w_xp_p[l, r, :, 64:64 + DS] = xp_t[:, DTR + DS:DTR + 2 * DS]
    m["w_xp"] = w_xp_p
    m["w_dt"] = np.ascontiguousarray(inp["dt_w"].transpose(0, 1, 3, 2))
    m["dtb"] = np.ascontiguousarray(
        inp["dt_b"].reshape(NM, 2, NICH, 128).transpose(0, 1, 3, 2))
    m["a_neg"] = np.ascontiguousarray(
        (-np.exp(inp["A_log"])).reshape(NM, 2, NICH, 128, DS)
        .transpose(0, 1, 3, 2, 4).reshape(NM, 2, 128, NICH * DS))
    dsk = inp["D_skip"]  # [NM,2,DI]
    dskd = np.zeros((NM, 2, 128, NICH * 128), np.float32)
    for l in range(NM):
        for r in range(2):
            for c in range(NICH):
                dskd[l, r, ii, c * 128 + ii] = dsk[l, r, c * 128:(c + 1) * 128]
    m["dskd"] = dskd
    m["w_out"] = np.ascontiguousarray(inp["out_w"].transpose(0, 1, 3, 2))
    m["g_bc"] = np.ascontiguousarray(np.broadcast_to(inp["cln_gamma"],
                                                     (128, D)))
    m["be_bc"] = np.ascontiguousarray(np.broadcast_to(inp["cln_beta"],
                                                      (128, D)))
    m["id128"] = np.eye(128, dtype=np.float32)
    t_of = np.arange(T, dtype=np.int64).reshape(NTB, 128).T  # [128, NTB]
    m["idx"] = np.ascontiguousarray(((T - 1) - t_of).astype(np.int32))

    import ml_dtypes
    a_fm_all, v_fm_all = m["a_fm"], m["v_fm"]
    a_tmb_all = m["a_tmb"]
    blobs = []
    for ci in range(N_CORES):
        m["a_fm"] = a_fm_all[ci:ci + 1]
        m["v_fm"] = v_fm_all[ci:ci + 1]
        m["a_tmb"] = a_tmb_all[ci:ci + 1]
        parts = []
        for name, shape, dt in PACK:
            a = np.ascontiguousarray(m[name])
            assert a.shape == tuple(shape), (name, a.shape, shape)
            if dt == "h":
                a = a.astype(ml_dtypes.bfloat16).ravel().view(np.float32)
            elif dt == "i":
                a = a.view(np.float32).ravel()
            else:
                a = a.ravel()
            parts.append(a)
        blobs.append({"blob": np.concatenate(parts)})
    return blobs


def get_nc(**kw):
    key = ("nc", tuple(sorted(kw.items())))
    if key not in _CACHE:
        _CACHE[key] = build_nc(**kw)
    return _CACHE[key]


def kernel(**inputs) -> np.ndarray:
    nc = get_nc()
    maps = make_in_maps(inputs)
    res = run_bass_kernel_spmd(nc, maps, list(range(N_CORES)))
    out = np.stack([res.results[i]["out"].reshape(T, D)
                    for i in range(N_CORES)])
    return out.astype(np.float32)


if __name__ == "__main__":
    import reference
    inputs = {k: np.asarray(v) for k, v in reference.setup_inputs().items()}
    got = kernel(**inputs)
    print("kernel ran; out shape", got.shape)


# revision 11
# speedup vs baseline: 3.8864x; 1.0406x over previous
"""DeepFusionCrossMamba Trainium2 kernel, v3.

Data-parallel over batch: 4 cores x 1 batch (the sharding hint).
Each core runs a single-batch program; the host test harness drives the
four cores with independently AOT-compiled single-device executables
dispatched from persistent threads (the shard_map multi-device path
costs ~4ms/call of dispatch; four independent dispatches overlap to
~1.6ms). Engine assignment uses measured HW rates (DVE 1.07/0.54
ns/elem f32/bf16, ACT 0.87, GPS 2.15, matmul ~0.84us per [128,512])
and the measured fact that GPSIMD serializes against DVE (shared SBUF
ports) while ACT/PE overlap DVE freely:

- DVE: scans (sp=4 packing), dBu multiplies (bf16 2x, broadcast-read u),
  gate mults, rmsnorm apply.
- GPS: hc = hsc*cm (bf16), u, residual adds, psum->sbuf bulk copies.
- PE:  all matmuls incl. depthwise conv as 4 accumulating diag matmuls
  (diagonalized conv + D_skip weights precomputed on host).
- ACT: exps (function-grouped to avoid 1.3us table reloads), softplus,
  silus, small psum->sbuf copies. No DMAs on the ACT queue.
- SP(sync): every dma_start + dma_start_transpose (feature<->time-major
  transposes moved off the PE).
"""

import numpy as np

import concourse.bass as bass
import concourse.bacc as bacc
import concourse.mybir as mybir
import concourse.tile as tile
from concourse.bass import IndirectOffsetOnAxis
from concourse.bass_utils import run_bass_kernel_spmd

F32 = mybir.dt.float32
BF16 = mybir.dt.bfloat16
I32 = mybir.dt.int32
AF = mybir.ActivationFunctionType
OP = mybir.AluOpType

B, T, D = 4, 1024, 256
NM, DI, DS, DCONV, DTR = 2, 512, 16, 4, 16
NDCH = D // 128    # 2 feature chunks of d_model
NICH = DI // 128   # 4 feature chunks of d_inner
NTB = T // 128     # 8 time blocks
SP = 4             # s-values packed per scan instruction
NG = DS // SP      # s-groups
EPS = 1e-8
NBPC = 1           # batches per core
N_CORES = 4

_CACHE = {}

PACK = [
    ("a_fm", [NBPC, D, T], "h"),
    ("a_tmb", [NBPC, 128, NTB * D], "h"),
    ("v_fm", [NBPC, D, T], "h"),
    ("w_a2v", [D, D], "h"),
    ("b_a2v", [128, NDCH], "f"),
    ("w_v2a", [D, D], "h"),
    ("b_v2a", [128, NDCH], "f"),
    ("pk", [3, D, D], "h"),
    ("bn_s", [128, NDCH], "f"),
    ("bn_b", [128, NDCH], "f"),
    ("rmsw_bc", [NM, 128, D], "f"),
    ("w_in", [NM, 2, D, 2 * DI], "h"),
    ("cwd", [NM, 2, 128, NICH * DCONV * 128], "h"),  # diag(conv taps), row-major
    ("cb", [NM, 2, 128, NICH], "f"),
    ("w_xp", [NM, 2, DI, 80], "h"),
    ("w_dt", [NM, 2, DTR, DI], "h"),
    ("dtb", [NM, 2, 128, NICH], "f"),
    ("a_neg", [NM, 2, 128, NICH * DS], "f"),
    ("dskd", [NM, 2, 128, NICH * 128], "h"),         # diag(D_skip)
    ("w_out", [NM, 2, DI, D], "h"),
    ("g_bc", [128, D], "f"),
    ("be_bc", [128, D], "f"),
    ("id128", [128, 128], "h"),
    ("rev128", [128, 128], "h"),
    ("idx", [128, NTB], "i"),
]


def _pack_offsets():
    offs, off = {}, 0
    for name, shape, dt in PACK:
        n = 1
        for s in shape:
            n *= s
        if dt == "h":
            assert n % 2 == 0
            n //= 2
        offs[name] = (off, list(shape), dt)
        off += n
    return offs, off


OFFS, BLOB_N = _pack_offsets()


def _decl(nc, name, shape, dtype=F32, out=False):
    return nc.declare_dram_parameter(name, list(shape), dtype, isOutput=out)


def build_nc(nlayers=NM, nbatches=NBPC, probe=()):
    nc = bacc.Bacc(None, target_bir_lowering=False, debug=False)
    blob_d = _decl(nc, "blob", [BLOB_N])
    out_d = _decl(nc, "out", [NBPC * T, D], BF16, out=True)
    with tile.TileContext(nc) as tc:
        _body(nc, tc, nlayers, nbatches, blob_d, out_d, probe)
    nc.finalize()
    return nc


def _body(nc, tc, nlayers, nbatches, blob_d, out_d, probe=()):
    from contextlib import ExitStack
    ctx = ExitStack()
    with ctx:
        perm = ctx.enter_context(tc.tile_pool(name="perm", bufs=1))
        pwt = ctx.enter_context(tc.tile_pool(name="pwt", bufs=2))
        pwork = ctx.enter_context(tc.tile_pool(name="pwork", bufs=1))
        pscan = ctx.enter_context(tc.tile_pool(name="pscan", bufs=2))
        pbc = ctx.enter_context(tc.tile_pool(name="pbc", bufs=2))
        pdbu = ctx.enter_context(tc.tile_pool(name="pdbu", bufs=1))
        psmall = ctx.enter_context(tc.tile_pool(name="psmall", bufs=2))
        ptrans = ctx.enter_context(tc.tile_pool(name="ptrans", bufs=2,
                                                space="PSUM"))
        ppsy = ctx.enter_context(tc.tile_pool(name="ppsy", bufs=1,
                                              space="PSUM"))
        ppso = ctx.enter_context(tc.tile_pool(name="ppso", bufs=1,
                                              space="PSUM"))
        ppst = ctx.enter_context(tc.tile_pool(name="ppst", bufs=2,
                                              space="PSUM"))

        th_sl = [slice(0, 512), slice(512, 1024)]
        blob_ap = blob_d[:]

        def bsl(name, *pre, rows=None):
            off, shape, dt = OFFS[name]
            div = 2 if dt == "h" else 1
            for i, ix in enumerate(pre):
                stride = 1
                for s in shape[i + 1:]:
                    stride *= s
                off += ix * stride // div
            s = shape[len(pre):]
            assert len(s) == 2
            r0, r1 = (0, s[0]) if rows is None else rows
            off += r0 * s[1] // div
            n = (r1 - r0) * s[1] // div
            ap = blob_ap[off:off + n].rearrange("(a b) -> a b", a=r1 - r0)
            if dt == "h":
                ap = ap.bitcast(BF16)
            elif dt == "i":
                ap = ap.bitcast(I32)
            return ap

        def load(dram, shape, name, dtype=F32, pool=perm):
            if not isinstance(dram, bass.AP):
                dram = dram[:]
            t = pool.tile(shape, dtype, tag=name, name=name)
            nc.sync.dma_start(t[:], dram)
            return t

        # ---------------- shared persistent loads ----------------
        w_a2v = [load(bsl("w_a2v", rows=(c * 128, (c + 1) * 128)), [128, D],
                      f"w_a2v{c}", BF16) for c in range(NDCH)]
        w_v2a = [load(bsl("w_v2a", rows=(c * 128, (c + 1) * 128)), [128, D],
                      f"w_v2a{c}", BF16) for c in range(NDCH)]
        b_a2v = load(bsl("b_a2v"), [128, NDCH], "b_a2v")
        b_v2a = load(bsl("b_v2a"), [128, NDCH], "b_v2a")
        pk = [[load(bsl("pk", k, rows=(c * 128, (c + 1) * 128)), [128, D],
                    f"pk{k}{c}", BF16)
               for c in range(NDCH)] for k in range(3)]
        bn_s = load(bsl("bn_s"), [128, NDCH], "bn_s")
        bn_b = load(bsl("bn_b"), [128, NDCH], "bn_b")
        g_bc = load(bsl("g_bc"), [128, D], "g_bc")
        be_bc = load(bsl("be_bc"), [128, D], "be_bc")
        idx = load(bsl("idx"), [128, NTB], "idx", I32)
        id128b = load(bsl("id128"), [128, 128], "id128b", BF16)
        rev128b = load(bsl("rev128"), [128, 128], "rev128", BF16)
        rmsw = [load(bsl("rmsw_bc", l), [128, D], f"rmsw{l}")
                for l in range(nlayers)]

        dAp = []
        for i in range(2):
            t = perm.tile([128, SP * T], BF16, tag=f"dA{i}", name=f"dA{i}")
            nc.vector.memset(t[:, 0:SP * T:T], 0.0)
            dAp.append(t)

        bounceY = [nc.dram_tensor(f"bounceY{i}", [T, D], BF16)
                   for i in range(2)]
        xdbl_dram = nc.dram_tensor("xdbl_dram", [32, T], BF16)

        def brd(src_rows_ap, n):
            # broadcast n DRAM rows of length T to 128 partitions
            return src_rows_ap.rearrange("a b -> (a b)").partition_broadcast(128)

        for bi in range(nbatches):
            # ================ preamble (feature-major) ================
            a_fm = [load(bsl("a_fm", bi, rows=(c * 128, (c + 1) * 128)),
                         [128, T], f"a_fm{c}", BF16, pool=pwork)
                    for c in range(NDCH)]
            # a_tm[j-block layout]: time-major audio direct from blob
            a_tm = load(bsl("a_tmb", bi), [128, NTB * D], "a_tm", BF16)
            g_a2v = [pwork.tile([128, T], BF16, tag=f"gate{c}",
                                name=f"g_a2v{c}") for c in range(NDCH)]
            for ec in range(NDCH):
                for th in range(2):
                    ps = ptrans.tile([128, 512], F32, tag="ps", name="ps")
                    for dc in range(NDCH):
                        nc.tensor.matmul(ps[:],
                                         w_a2v[dc][:, ec * 128:(ec + 1) * 128],
                                         a_fm[dc][:, th_sl[th]],
                                         start=(dc == 0), stop=(dc == NDCH - 1))
                    nc.scalar.activation(g_a2v[ec][:, th_sl[th]], ps[:],
                                         AF.Sigmoid, bias=b_a2v[:, ec:ec + 1])
            v_fm = [load(bsl("v_fm", bi, rows=(c * 128, (c + 1) * 128)),
                         [128, T], f"a_fm{c}", BF16, pool=pwork)
                    for c in range(NDCH)]
            v_pad = [pwork.tile([128, T + 3], BF16, tag=f"xc_pad{c}",
                                name=f"v_pad{c}") for c in range(NDCH)]
            for c in range(NDCH):
                nc.vector.memset(v_pad[c][:, 0:1], 0.0)
                nc.vector.memset(v_pad[c][:, T + 1:T + 2], 0.0)
                nc.gpsimd.tensor_tensor(out=v_pad[c][:, 1:T + 1],
                                        in0=v_fm[c][:], in1=g_a2v[c][:],
                                        op=OP.mult)
            g_v2a = [pwork.tile([128, T], BF16, tag=f"gate{c+2}",
                                name=f"g_v2a{c}") for c in range(NDCH)]
            for ec in range(NDCH):
                for th in range(2):
                    ps = ptrans.tile([128, 512], F32, tag="ps", name="ps")
                    for dc in range(NDCH):
                        nc.tensor.matmul(ps[:],
                                         w_v2a[dc][:, ec * 128:(ec + 1) * 128],
                                         v_pad[dc][:, th * 512 + 1:
                                                    th * 512 + 513],
                                         start=(dc == 0), stop=(dc == NDCH - 1))
                    nc.scalar.activation(g_v2a[ec][:, th_sl[th]], ps[:],
                                         AF.Sigmoid, bias=b_v2a[:, ec:ec + 1])
            dlt = [pwork.tile([128, T], BF16, tag=f"xcs{c}", name=f"dlt{c}")
                   for c in range(NDCH)]
            for ec in range(NDCH):
                for th in range(2):
                    ps = ptrans.tile([128, 512], F32, tag="ps", name="ps")
                    first = True
                    for k in range(3):
                        for dc in range(NDCH):
                            nc.tensor.matmul(
                                ps[:], pk[k][dc][:, ec * 128:(ec + 1) * 128],
                                v_pad[dc][:, th * 512 + k: th * 512 + k + 512],
                                start=first, stop=(k == 2 and dc == NDCH - 1))
                            first = False
                    nc.scalar.activation(dlt[ec][:, th_sl[th]], ps[:],
                                         AF.Gelu, bias=bn_b[:, ec:ec + 1],
                                         scale=bn_s[:, ec:ec + 1])
            gdlt = [pwork.tile([128, T], BF16, tag=f"xcs{c+2}",
                               name=f"gdlt{c}") for c in range(NDCH)]
            for c in range(NDCH):
                nc.gpsimd.tensor_tensor(out=gdlt[c][:], in0=g_v2a[c][:],
                                        in1=dlt[c][:], op=OP.mult)
            # x0 (time-major f32) = a_tm + transpose(gdlt)
            gd_tm = pwork.tile([128, NTB * D], BF16, tag="h_tm",
                               name="gd_tm")
            for j in range(NTB):
                for dc in range(NDCH):
                    psT = ppst.tile([128, 128], BF16, tag="pst",
                                    name="pst")
                    nc.tensor.transpose(psT[:],
                                        gdlt[dc][:, j * 128:(j + 1) * 128],
                                        id128b[:])
                    nc.scalar.copy(
                        gd_tm[:, j * D + dc * 128: j * D + (dc + 1) * 128],
                        psT[:])
            x_tm = perm.tile([128, NTB * D], BF16, tag="x_tm0",
                             name="x_tm0")
            nc.vector.tensor_tensor(out=x_tm[:], in0=a_tm[:], in1=gd_tm[:],
                                    op=OP.add)

            # ================ mamba layers ================
            for l in range(nlayers):
                # rmsnorm over channel dim
                st = psmall.tile([128, NTB], F32, tag="st", name="st")
                sq = pwork.tile([128, D], F32, tag="sq", name="sq")
                for j in range(NTB):
                    nc.scalar.activation(sq[:], x_tm[:, j * D:(j + 1) * D],
                                         AF.Square, accum_out=st[:, j:j + 1])
                ms = psmall.tile([128, NTB], F32, tag="ms", name="ms")
                nc.vector.tensor_scalar(out=ms[:], in0=st[:],
                                        scalar1=1.0 / D, scalar2=1e-5,
                                        op0=OP.mult, op1=OP.add)
                msr = psmall.tile([128, NTB], F32, tag="msr", name="msr")
                nc.vector.reciprocal(out=msr[:], in_=ms[:])
                rstd = psmall.tile([128, NTB], F32, tag="rstd", name="rstd")
                nc.scalar.activation(rstd[:], msr[:], AF.Sqrt)
                h_tm = pwork.tile([128, NTB * D], BF16, tag="h_tm",
                                  name="h_tm")
                for j in range(NTB):
                    nc.vector.scalar_tensor_tensor(
                        out=h_tm[:, j * D:(j + 1) * D],
                        in0=x_tm[:, j * D:(j + 1) * D],
                        scalar=rstd[:, j:j + 1], in1=rmsw[l][:],
                        op0=OP.mult, op1=OP.mult)
                h_fm = [pwork.tile([128, T], BF16, tag=f"h_fm{c}",
                                   name=f"h_fm{c}") for c in range(NDCH)]
                for j in range(NTB):
                    for dc in range(NDCH):
                        psT = ppst.tile([128, 128], BF16, tag="pst",
                                        name="pst")
                        nc.tensor.transpose(
                            psT[:],
                            h_tm[:, j * D + dc * 128: j * D + (dc + 1) * 128],
                            id128b[:])
                        nc.scalar.copy(h_fm[dc][:, j * 128:(j + 1) * 128],
                                       psT[:])
                h_rev_fm = [h_fm[c][:, ::-1] for c in range(NDCH)]

                ytm = [None, None]
                x_new = None
                for r in range(2):
                    hsrc = h_fm if r == 0 else h_rev_fm
                    w_in_l = [load(bsl("w_in", l, r,
                                       rows=(c * 128, (c + 1) * 128)),
                                   [128, 2 * DI], f"w_in{c}", BF16, pool=pwt)
                              for c in range(NDCH)]
                    cwd_t = load(bsl("cwd", l, r),
                                 [128, NICH * DCONV * 128], "cwd", BF16,
                                 pool=pwt)
                    cwd_l = [[cwd_t[:, (c * DCONV + k) * 128:
                                    (c * DCONV + k + 1) * 128]
                              for k in range(DCONV)] for c in range(NICH)]
                    cb_l = load(bsl("cb", l, r), [128, NICH], "cb", pool=pwt)
                    w_xp_l = [load(bsl("w_xp", l, r,
                                       rows=(c * 128, (c + 1) * 128)),
                                   [128, 80], f"w_xp{c}", BF16, pool=pwt)
                              for c in range(NICH)]
                    w_dt_l = load(bsl("w_dt", l, r), [DTR, DI], "w_dt",
                                  BF16, pool=pwt)
                    dtb_l = load(bsl("dtb", l, r), [128, NICH], "dtb",
                                 pool=pwt)
                    a_neg_l = load(bsl("a_neg", l, r), [128, NICH * DS],
                                   "a_neg", pool=pwt)
                    dskd_t = load(bsl("dskd", l, r), [128, NICH * 128],
                                  "dskd", BF16, pool=pwt)
                    dskd_l = [dskd_t[:, c * 128:(c + 1) * 128]
                              for c in range(NICH)]
                    w_out_l = [load(bsl("w_out", l, r,
                                        rows=(c * 128, (c + 1) * 128)),
                                    [128, D], f"w_out{c}", BF16, pool=pwt)
                               for c in range(NICH)]

                    # ---- in_proj -> xc (padded bf16), silu(z) ----
                    xc_pad = [pwork.tile([128, T + 3], BF16,
                                         tag=f"xc_pad{c}", name=f"xc_pad{c}")
                              for c in range(NICH)]
                    g_z = [pwork.tile([128, T], BF16, tag=f"gate{c}",
                                      name=f"g_z{c}") for c in range(NICH)]
                    for c in range(NICH):
                        nc.vector.memset(xc_pad[c][:, 0:3], 0.0)
                    for ec in range(2 * NICH):
                        for th in range(2):
                            ps = ptrans.tile([128, 512], F32, tag="ps",
                                             name="ps")
                            for dc in range(NDCH):
                                nc.tensor.matmul(
                                    ps[:],
                                    w_in_l[dc][:, ec * 128:(ec + 1) * 128],
                                    hsrc[dc][:, th_sl[th]],
                                    start=(dc == 0), stop=(dc == NDCH - 1))
                            if ec < NICH:
                                nc.scalar.copy(
                                    xc_pad[ec][:, 3 + th * 512:
                                               3 + th * 512 + 512], ps[:])
                            else:
                                nc.scalar.activation(
                                    g_z[ec - NICH][:, th_sl[th]], ps[:],
                                    AF.Silu)
                    # ---- depthwise conv: 4 accumulating diag matmuls ----
                    xc_s = [pwork.tile([128, T], BF16, tag=f"xcs{c}",
                                       name=f"xc_s{c}") for c in range(NICH)]
                    for c in range(NICH):
                        for th in range(2):
                            psc = ptrans.tile([128, 512], F32, tag="ps",
                                              name="ps")
                            for k in range(DCONV):
                                nc.tensor.matmul(
                                    psc[:], cwd_l[c][k],
                                    xc_pad[c][:, th * 512 + k:
                                              th * 512 + k + 512],
                                    start=(k == 0), stop=(k == DCONV - 1))
                            nc.scalar.activation(xc_s[c][:, th_sl[th]],
                                                 psc[:], AF.Silu,
                                                 bias=cb_l[:, c:c + 1])
                    # ---- x_proj -> xdbl rows [dt(16) | B(16) | C(16)] ----
                    xdbl = pwork.tile([DTR, T], BF16, tag="xdbl",
                                      name="xdbl")
                    xdbl_bf = pwork.tile([48, T], BF16, tag="xdbl_bf",
                                         name="xdbl_bf")
                    for th in range(2):
                        psx = ptrans.tile([80, 512], F32, tag="ps",
                                          name="ps")
                        for c in range(NICH):
                            nc.tensor.matmul(psx[:], w_xp_l[c][:],
                                             xc_s[c][:, th_sl[th]],
                                             start=(c == 0),
                                             stop=(c == NICH - 1))
                        nc.scalar.copy(xdbl[0:DTR, th_sl[th]], psx[0:DTR, :])
                        nc.scalar.copy(xdbl_bf[0:16, th_sl[th]],
                                       psx[32:48, :])
                        nc.scalar.copy(xdbl_bf[32:48, th_sl[th]],
                                       psx[64:80, :])
                        nc.sync.dma_start(xdbl_dram[0:16, th_sl[th]],
                                          xdbl_bf[0:16, th_sl[th]])
                        nc.sync.dma_start(xdbl_dram[16:32, th_sl[th]],
                                          xdbl_bf[32:48, th_sl[th]])

                    # ---- prep: delta (bf16), u (bf16), pipelined 2 ahead
                    # (delta/u rotate on c%2 tags; prep(c+2) is emitted
                    # after chunk c's exps so the ACT queue stays acyclic)
                    deltas, us = [None] * NICH, [None] * NICH

                    def prep(c):
                        psd = [ptrans.tile([128, 512], F32, tag="ps",
                                           name="ps") for _ in range(2)]
                        for th in range(2):
                            nc.tensor.matmul(psd[th][:],
                                             w_dt_l[:, c * 128:(c + 1) * 128],
                                             xdbl[0:DTR, th_sl[th]],
                                             start=True, stop=True)
                        esp = pwork.tile([128, T], BF16, tag="esp",
                                         name="esp")
                        for th in range(2):
                            nc.scalar.activation(esp[:, th_sl[th]],
                                                 psd[th][:], AF.Exp,
                                                 bias=dtb_l[:, c:c + 1])
                        delta = pwork.tile([128, T], BF16,
                                           tag=f"delta{c % 2}",
                                           name=f"delta{c % 2}")
                        nc.scalar.activation(delta[:], esp[:], AF.Ln,
                                             bias=1.0)
                        deltas[c] = delta
                        u = pwork.tile([128, T], BF16, tag=f"u{c % 2}",
                                       name=f"u{c % 2}")
                        nc.vector.tensor_tensor(out=u[:], in0=delta[:],
                                                in1=xc_s[c][:], op=OP.mult)
                        us[c] = u

                    prep(0)
                    prep(1)

                    # ---- scan section ----
                    y_g = [pwork.tile([128, T], BF16, tag=f"y_g{c}",
                                      name=f"y_g{c}") for c in range(NICH)]
                    for c in range(NICH):
                        psy = [ppsy.tile([128, 512], F32, tag=f"psy{th}",
                                         name=f"psy{th}") for th in range(2)]
                        for g in range(NG):
                            s0 = g * SP
                            dA = dAp[g % 2]
                            if "no_exp" not in probe:
                                for sv in range(SP):
                                    nc.scalar.activation(
                                        dA[:, sv * T + 1:(sv + 1) * T],
                                        deltas[c][:, 1:T], AF.Exp,
                                        scale=a_neg_l[:, c * DS + s0 + sv:
                                                      c * DS + s0 + sv + 1])
                            bm = pbc.tile([128, SP * T], BF16, tag="bm",
                                          name="bm")
                            nc.sync.dma_start(
                                bm[:], brd(xdbl_dram[s0:s0 + SP, :], SP))
                            cm = pbc.tile([128, SP * T], BF16, tag="cm",
                                          name="cm")
                            nc.sync.dma_start(
                                cm[:], brd(xdbl_dram[16 + s0:16 + s0 + SP, :],
                                           SP))
                            dBu = pdbu.tile([128, SP * T], BF16, tag="dBu",
                                            name="dBu")
                            if "no_dbu" not in probe:
                                nc.vector.tensor_tensor(
                                out=dBu[:].rearrange("p (a t) -> p a t",
                                                     a=SP),
                                in0=us[c][:].rearrange("p (a t) -> p a t",
                                                       a=1)
                                .broadcast_to([128, SP, T]),
                                    in1=bm[:].rearrange("p (a t) -> p a t",
                                                        a=SP),
                                    op=OP.mult)
                            hsc = pscan.tile([128, SP * T], BF16, tag="hsc",
                                             name="hsc")
                            if "scan_copy" in probe:
                                nc.vector.tensor_copy(out=hsc[:],
                                                      in_=dBu[:])
                            else:
                                nc.vector.tensor_tensor_scan(
                                    hsc[:], dA[:], dBu[:], 0.0,
                                    OP.mult, OP.add)
                            hc = pbc.tile([128, SP * T], BF16, tag="hc",
                                          name="hc")
                            heng = nc.gpsimd if "hc_gps" in probe \
                                else nc.vector
                            heng.tensor_tensor(out=hc[:], in0=hsc[:],
                                               in1=cm[:], op=OP.mult)
                            for sv in range(SP):
                                if "no_psy" in probe and not (g == 0 and sv == 0):
                                    continue
                                for th in range(2):
                                    nc.tensor.matmul(
                                        psy[th][:], id128b[:],
                                        hc[:, sv * T + th * 512:
                                           sv * T + th * 512 + 512],
                                        start=(g == 0 and sv == 0),
                                        stop=False)
                        for th in range(2):
                            nc.tensor.matmul(psy[th][:], dskd_l[c],
                                             xc_s[c][:, th_sl[th]],
                                             start=False, stop=True)
                        for th in range(2):
                            nc.vector.tensor_tensor(out=y_g[c][:, th_sl[th]],
                                                    in0=psy[th][:],
                                                    in1=g_z[c][:, th_sl[th]],
                                                    op=OP.mult)
                        if c + 2 < NICH:
                            prep(c + 2)

                    # ---- out_proj (time-major j-blocks); fwd residual
                    # add folds straight out of PSUM on DVE ----
                    if r == 0:
                        x_new = perm.tile([128, NTB * D], BF16,
                                          tag=f"x_tm{(l + 1) % 2}",
                                          name=f"x_tm{(l + 1) % 2}")
                    else:
                        ytm[1] = pwork.tile([128, NTB * D], BF16,
                                            tag="ytm1", name="ytm1")
                    for jh in range(2):
                        pso = ppso.tile([128, 4 * D], F32, tag="pso",
                                        name="pso")
                        for j4 in range(4):
                            j = jh * 4 + j4
                            for c in range(NICH):
                                nc.tensor.matmul(
                                    pso[:, j4 * D:(j4 + 1) * D],
                                    y_g[c][:, j * 128:(j + 1) * 128],
                                    w_out_l[c][:],
                                    start=(c == 0), stop=(c == NICH - 1))
                        if r == 0:
                            nc.vector.tensor_tensor(
                                out=x_new[:, jh * 4 * D:(jh + 1) * 4 * D],
                                in0=pso[:],
                                in1=x_tm[:, jh * 4 * D:(jh + 1) * 4 * D],
                                op=OP.add)
                        else:
                            nc.scalar.copy(
                                ytm[1][:, jh * 4 * D:(jh + 1) * 4 * D],
                                pso[:])

                # reverse bwd output on-chip: anti-diagonal permutation
                # matmul flips partitions; j-blocks are read mirrored
                artm = pwork.tile([128, NTB * D], BF16, tag="artm",
                                  name="artm")
                for j in range(NTB):
                    psR = ptrans.tile([128, D], F32, tag="ps", name="ps")
                    nc.tensor.matmul(
                        psR[:], rev128b[:],
                        ytm[1][:, (NTB - 1 - j) * D:(NTB - j) * D],
                        start=True, stop=True)
                    nc.scalar.copy(artm[:, j * D:(j + 1) * D], psR[:])
                nc.vector.tensor_tensor(out=x_new[:], in0=x_new[:],
                                        in1=artm[:], op=OP.add)
                x_tm = x_new

            # ================ final channel LayerNorm ================
            s_t = pwork.tile([128, NTB * D], BF16, tag="h_tm", name="s_t")
            nc.vector.tensor_tensor(out=s_t[:], in0=x_tm[:], in1=a_tm[:],
                                    op=OP.add)
            stm = psmall.tile([128, NTB], F32, tag="stm", name="stm")
            stv = psmall.tile([128, NTB], F32, tag="stv", name="stv")
            dump = pwork.tile([128, D], F32, tag="sq", name="dump")
            for j in range(NTB):
                nc.scalar.activation(dump[:], s_t[:, j * D:(j + 1) * D],
                                     AF.Copy, accum_out=stm[:, j:j + 1])
                nc.scalar.activation(dump[:], s_t[:, j * D:(j + 1) * D],
                                     AF.Square, accum_out=stv[:, j:j + 1])
            mu = psmall.tile([128, NTB], F32, tag="mu", name="mu")
            nc.vector.tensor_scalar(out=mu[:], in0=stm[:], scalar1=1.0 / D,
                                    scalar2=None, op0=OP.mult)
            var = psmall.tile([128, NTB], F32, tag="var", name="var")
            nc.vector.tensor_scalar(out=var[:], in0=stv[:], scalar1=1.0 / D,
                                    scalar2=None, op0=OP.mult)
            mu2 = psmall.tile([128, NTB], F32, tag="mu2", name="mu2")
            nc.vector.tensor_tensor(out=mu2[:], in0=mu[:], in1=mu[:],
                                    op=OP.mult)
            nc.vector.tensor_tensor(out=var[:], in0=var[:], in1=mu2[:],
                                    op=OP.subtract)
            ve = psmall.tile([128, NTB], F32, tag="ve", name="ve")
            nc.vector.tensor_scalar(out=ve[:], in0=var[:], scalar1=EPS,
                                    scalar2=None, op0=OP.add)
            vr = psmall.tile([128, NTB], F32, tag="vr", name="vr")
            nc.vector.reciprocal(out=vr[:], in_=ve[:])
            rstd2 = psmall.tile([128, NTB], F32, tag="rstd2", name="rstd2")
            nc.scalar.activation(rstd2[:], vr[:], AF.Sqrt)
            otm = pwork.tile([128, NTB * D], BF16, tag="gd_out", name="otm")
            for j in range(NTB):
                sl = slice(j * D, (j + 1) * D)
                nc.vector.tensor_scalar(out=otm[:, sl], in0=s_t[:, sl],
                                        scalar1=mu[:, j:j + 1],
                                        scalar2=rstd2[:, j:j + 1],
                                        op0=OP.subtract, op1=OP.mult)
            otm3 = otm[:].rearrange("p (j d) -> p j d", j=NTB)
            nc.vector.tensor_tensor(out=otm3, in0=otm3,
                                    in1=g_bc[:].rearrange(
                                        "p (a d) -> p a d", a=1)
                                    .broadcast_to([128, NTB, D]),
                                    op=OP.mult)
            nc.vector.tensor_tensor(out=otm3, in0=otm3,
                                    in1=be_bc[:].rearrange(
                                        "p (a d) -> p a d", a=1)
                                    .broadcast_to([128, NTB, D]),
                                    op=OP.add)
            nc.sync.dma_start(
                out_d[bi * T:(bi + 1) * T, :]
                .rearrange("(j p) d -> p j d", p=128),
                otm[:].rearrange("p (j d) -> p j d", j=NTB))


# ---------------- host side ----------------

def make_in_maps(inputs, num_cores=None):
    # one map per core; core i owns batch i (num_cores arg kept for
    # interface compatibility, the mesh is always N_CORES wide)
    inp = {k: np.asarray(v, dtype=np.float32) for k, v in inputs.items()}
    m = {}
    m["a_fm"] = np.ascontiguousarray(inp["audio"].transpose(0, 2, 1))
    # time-major j-block layout: a_tmb[b, p, j*D + d] = audio[b, j*128+p, d]
    m["a_tmb"] = np.ascontiguousarray(
        inp["audio"].reshape(B, NTB, 128, D).transpose(0, 2, 1, 3)
        .reshape(B, 128, NTB * D))
    m["v_fm"] = np.ascontiguousarray(inp["video"].transpose(0, 2, 1))
    m["w_a2v"] = np.ascontiguousarray(inp["gate_a2v_w"].T)
    m["b_a2v"] = np.ascontiguousarray(inp["gate_a2v_b"].reshape(NDCH, 128).T)
    m["w_v2a"] = np.ascontiguousarray(inp["gate_v2a_w"].T)
    m["b_v2a"] = np.ascontiguousarray(inp["gate_v2a_b"].reshape(NDCH, 128).T)
    m["pk"] = np.ascontiguousarray(
        np.stack([inp["proj_w"][:, :, k].T for k in range(3)]))
    m["bn_s"] = np.ascontiguousarray(
        (inp["bn_gamma"] / np.sqrt(1.0 + 1e-5)).reshape(NDCH, 128).T)
    m["bn_b"] = np.ascontiguousarray(inp["bn_beta"].reshape(NDCH, 128).T)
    m["rmsw_bc"] = np.ascontiguousarray(
        np.broadcast_to(inp["rms_w"][:, None, :], (NM, 128, D)))
    m["w_in"] = np.ascontiguousarray(inp["in_proj_w"].transpose(0, 1, 3, 2))
    # conv taps as diagonal matrices: cwd[l,r,c,k] = diag(conv_w[l,r,c*128:(c+1)*128,k])
    cw = inp["conv_w"]  # [NM,2,DI,DCONV]
    cwd = np.zeros((NM, 2, 128, NICH * DCONV * 128), np.float32)
    ii = np.arange(128)
    for l in range(NM):
        for r in range(2):
            for c in range(NICH):
                for k in range(DCONV):
                    cwd[l, r, ii, (c * DCONV + k) * 128 + ii] = \
                        cw[l, r, c * 128:(c + 1) * 128, k]
    m["cwd"] = cwd
    m["cb"] = np.ascontiguousarray(
        inp["conv_b"].reshape(NM, 2, NICH, 128).transpose(0, 1, 3, 2))
    w_xp_p = np.zeros((NM, 2, DI, 80), np.float32)
    for l in range(NM):
        for r in range(2):
            xp_t = inp["x_proj_w"][l, r].T  # [DI, 48]
            w_xp_p[l, r, :, 0:DTR] = xp_t[:, 0:DTR]
            w_xp_p[l, r, :, 32:32 + DS] = xp_t[:, DTR:DTR + DS]
            w_xp_p[l, r, :, 64:64 + DS] = xp_t[:, DTR + DS:DTR + 2 * DS]
    m["w_xp"] = w_xp_p
    m["w_dt"] = np.ascontiguousarray(inp["dt_w"].transpose(0, 1, 3, 2))
    m["dtb"] = np.ascontiguousarray(
        inp["dt_b"].reshape(NM, 2, NICH, 128).transpose(0, 1, 3, 2))
    m["a_neg"] = np.ascontiguousarray(
        (-np.exp(inp["A_log"])).reshape(NM, 2, NICH, 128, DS)
        .transpose(0, 1, 3, 2, 4).reshape(NM, 2, 128, NICH * DS))
    dsk = inp["D_skip"]  # [NM,2,DI]
    dskd = np.zeros((NM, 2, 128, NICH * 128), np.float32)
    for l in range(NM):
        for r in range(2):
            for c in range(NICH):
                dskd[l, r, ii, c * 128 + ii] = dsk[l, r, c * 128:(c + 1) * 128]
    m["dskd"] = dskd
    m["w_out"] = np.ascontiguousarray(inp["out_w"].transpose(0, 1, 3, 2))
    m["g_bc"] = np.ascontiguousarray(np.broadcast_to(inp["cln_gamma"],
                                                     (128, D)))
    m["be_bc"] = np.ascontiguousarray(np.broadcast_to(inp["cln_beta"],
                                                      (128, D)))
    m["id128"] = np.eye(128, dtype=np.float32)
    m["rev128"] = np.eye(128, dtype=np.float32)[::-1].copy()
    t_of = np.arange(T, dtype=np.int64).reshape(NTB, 128).T  # [128, NTB]
    m["idx"] = np.ascontiguousarray(((T - 1) - t_of).astype(np.int32))

    import ml_dtypes
    a_fm_all, v_fm_all = m["a_fm"], m["v_fm"]
    a_tmb_all = m["a_tmb"]
    blobs = []
    for ci in range(N_CORES):
        m["a_fm"] = a_fm_all[ci:ci + 1]
        m["v_fm"] = v_fm_all[ci:ci + 1]
        m["a_tmb"] = a_tmb_all[ci:ci + 1]
        parts = []
        for name, shape, dt in PACK:
            a = np.ascontiguousarray(m[name])
            assert a.shape == tuple(shape), (name, a.shape, shape)
            if dt == "h":
                a = a.astype(ml_dtypes.bfloat16).ravel().view(np.float32)
            elif dt == "i":
                a = a.view(np.float32).ravel()
            else:
                a = a.ravel()
            parts.append(a)
        blobs.append({"blob": np.concatenate(parts)})
    return blobs


def get_nc(**kw):
    key = ("nc", tuple(sorted(kw.items())))
    if key not in _CACHE:
        _CACHE[key] = build_nc(**kw)
    return _CACHE[key]


def kernel(**inputs) -> np.ndarray:
    nc = get_nc()
    maps = make_in_maps(inputs)
    res = run_bass_kernel_spmd(nc, maps, list(range(N_CORES)))
    out = np.stack([res.results[i]["out"].reshape(T, D)
                    for i in range(N_CORES)])
    return out.astype(np.float32)


if __name__ == "__main__":
    import reference
    inputs = {k: np.asarray(v) for k, v in reference.setup_inputs().items()}
    got = kernel(**inputs)
    print("kernel ran; out shape", got.shape)


# revision 12
# speedup vs baseline: 4.2251x; 1.0872x over previous
"""DeepFusionCrossMamba Trainium2 kernel, v3.

Data-parallel over batch: 4 cores x 1 batch (the sharding hint).
Each core runs a single-batch program; the host test harness drives the
four cores with independently AOT-compiled single-device executables
dispatched from persistent threads (the shard_map multi-device path
costs ~4ms/call of dispatch; four independent dispatches overlap to
~1.6ms). Engine assignment uses measured HW rates (DVE 1.07/0.54
ns/elem f32/bf16, ACT 0.87, GPS 2.15, matmul ~0.84us per [128,512])
and the measured fact that GPSIMD serializes against DVE (shared SBUF
ports) while ACT/PE overlap DVE freely:

- DVE: scans (sp=4 packing), dBu multiplies (bf16 2x, broadcast-read u),
  hc=h*C mults, u, gate mults, rmsnorm/residual adds. dA boundary zeros
  live in two persistent buffers memset once per call (a per-group
  memset would chain DVE->ACT->DVE and serialize the scan loop).
- GPSIMD: only preamble gating products (never concurrent with scans).
- PE: all matmuls; depthwise conv and D_skip as accumulating diag
  matmuls (diagonalized weights precomputed on host); h_tm->h_fm
  transposes; bwd-output time reversal via an anti-diagonal permutation
  matmul (replaces a DRAM bounce + indirect gathers).
- ACT: exps (function-grouped: each activation-table switch costs
  ~1.3us), softplus, silus, psum->sbuf copies. No DMAs on this queue.
- SP(sync): all dma_starts; bm/cm broadcast tiles are double-buffered so
  their WAR waits never clog the in-order DMA queue.
"""

import numpy as np

import concourse.bass as bass
import concourse.bacc as bacc
import concourse.mybir as mybir
import concourse.tile as tile
from concourse.bass import IndirectOffsetOnAxis
from concourse.bass_utils import run_bass_kernel_spmd

F32 = mybir.dt.float32
BF16 = mybir.dt.bfloat16
I32 = mybir.dt.int32
AF = mybir.ActivationFunctionType
OP = mybir.AluOpType

B, T, D = 4, 1024, 256
NM, DI, DS, DCONV, DTR = 2, 512, 16, 4, 16
NDCH = D // 128    # 2 feature chunks of d_model
NICH = DI // 128   # 4 feature chunks of d_inner
NTB = T // 128     # 8 time blocks
SP = 4             # s-values packed per scan instruction
NG = DS // SP      # s-groups
EPS = 1e-8
NBPC = 1           # batches per core
N_CORES = 4

_CACHE = {}

PACK = [
    ("a_fm", [NBPC, D, T], "h"),
    ("a_tmb", [NBPC, 128, NTB * D], "h"),
    ("v_fm", [NBPC, D, T], "h"),
    ("w_a2v", [D, D], "h"),
    ("b_a2v", [128, NDCH], "f"),
    ("w_v2a", [D, D], "h"),
    ("b_v2a", [128, NDCH], "f"),
    ("pk", [3, D, D], "h"),
    ("bn_s", [128, NDCH], "f"),
    ("bn_b", [128, NDCH], "f"),
    ("rmsw_bc", [NM, 128, D], "f"),
    ("w_in", [NM, 2, D, 2 * DI], "h"),
    ("cwd", [NM, 2, 128, NICH * DCONV * 128], "h"),  # diag(conv taps), row-major
    ("cb", [NM, 2, 128, NICH], "f"),
    ("w_xp", [NM, 2, DI, 80], "h"),
    ("w_dt", [NM, 2, DTR, DI], "h"),
    ("dtb", [NM, 2, 128, NICH], "f"),
    ("a_neg", [NM, 2, 128, NICH * DS], "f"),
    ("dskd", [NM, 2, 128, NICH * 128], "h"),         # diag(D_skip)
    ("w_out", [NM, 2, DI, D], "h"),
    ("g_bc", [128, D], "f"),
    ("be_bc", [128, D], "f"),
    ("id128", [128, 128], "h"),
    ("rev128", [128, 128], "h"),
    ("idx", [128, NTB], "i"),
]


def _pack_offsets():
    offs, off = {}, 0
    for name, shape, dt in PACK:
        n = 1
        for s in shape:
            n *= s
        if dt == "h":
            assert n % 2 == 0
            n //= 2
        offs[name] = (off, list(shape), dt)
        off += n
    return offs, off


OFFS, BLOB_N = _pack_offsets()


def _decl(nc, name, shape, dtype=F32, out=False):
    return nc.declare_dram_parameter(name, list(shape), dtype, isOutput=out)


def build_nc(nlayers=NM, nbatches=NBPC, probe=()):
    nc = bacc.Bacc(None, target_bir_lowering=False, debug=False)
    blob_d = _decl(nc, "blob", [BLOB_N])
    out_d = _decl(nc, "out", [NBPC * T, D], BF16, out=True)
    with tile.TileContext(nc) as tc:
        _body(nc, tc, nlayers, nbatches, blob_d, out_d, probe)
    nc.finalize()
    return nc


def _body(nc, tc, nlayers, nbatches, blob_d, out_d, probe=()):
    from contextlib import ExitStack
    ctx = ExitStack()
    with ctx:
        perm = ctx.enter_context(tc.tile_pool(name="perm", bufs=1))
        pwt = ctx.enter_context(tc.tile_pool(name="pwt", bufs=2))
        pwork = ctx.enter_context(tc.tile_pool(name="pwork", bufs=1))
        pscan = ctx.enter_context(tc.tile_pool(name="pscan", bufs=2))
        pbc = ctx.enter_context(tc.tile_pool(name="pbc", bufs=2))
        pdbu = ctx.enter_context(tc.tile_pool(name="pdbu", bufs=1))
        psmall = ctx.enter_context(tc.tile_pool(name="psmall", bufs=2))
        ptrans = ctx.enter_context(tc.tile_pool(name="ptrans", bufs=2,
                                                space="PSUM"))
        ppsy = ctx.enter_context(tc.tile_pool(name="ppsy", bufs=1,
                                              space="PSUM"))
        ppso = ctx.enter_context(tc.tile_pool(name="ppso", bufs=1,
                                              space="PSUM"))
        ppst = ctx.enter_context(tc.tile_pool(name="ppst", bufs=2,
                                              space="PSUM"))

        th_sl = [slice(0, 512), slice(512, 1024)]
        blob_ap = blob_d[:]

        def bsl(name, *pre, rows=None):
            off, shape, dt = OFFS[name]
            div = 2 if dt == "h" else 1
            for i, ix in enumerate(pre):
                stride = 1
                for s in shape[i + 1:]:
                    stride *= s
                off += ix * stride // div
            s = shape[len(pre):]
            assert len(s) == 2
            r0, r1 = (0, s[0]) if rows is None else rows
            off += r0 * s[1] // div
            n = (r1 - r0) * s[1] // div
            ap = blob_ap[off:off + n].rearrange("(a b) -> a b", a=r1 - r0)
            if dt == "h":
                ap = ap.bitcast(BF16)
            elif dt == "i":
                ap = ap.bitcast(I32)
            return ap

        def load(dram, shape, name, dtype=F32, pool=perm):
            if not isinstance(dram, bass.AP):
                dram = dram[:]
            t = pool.tile(shape, dtype, tag=name, name=name)
            nc.sync.dma_start(t[:], dram)
            return t

        # ---------------- shared persistent loads ----------------
        w_a2v = [load(bsl("w_a2v", rows=(c * 128, (c + 1) * 128)), [128, D],
                      f"w_a2v{c}", BF16) for c in range(NDCH)]
        w_v2a = [load(bsl("w_v2a", rows=(c * 128, (c + 1) * 128)), [128, D],
                      f"w_v2a{c}", BF16) for c in range(NDCH)]
        b_a2v = load(bsl("b_a2v"), [128, NDCH], "b_a2v")
        b_v2a = load(bsl("b_v2a"), [128, NDCH], "b_v2a")
        pk = [[load(bsl("pk", k, rows=(c * 128, (c + 1) * 128)), [128, D],
                    f"pk{k}{c}", BF16)
               for c in range(NDCH)] for k in range(3)]
        bn_s = load(bsl("bn_s"), [128, NDCH], "bn_s")
        bn_b = load(bsl("bn_b"), [128, NDCH], "bn_b")
        g_bc = load(bsl("g_bc"), [128, D], "g_bc")
        be_bc = load(bsl("be_bc"), [128, D], "be_bc")
        idx = load(bsl("idx"), [128, NTB], "idx", I32)
        id128b = load(bsl("id128"), [128, 128], "id128b", BF16)
        rev128b = load(bsl("rev128"), [128, 128], "rev128", BF16)
        rmsw = [load(bsl("rmsw_bc", l), [128, D], f"rmsw{l}")
                for l in range(nlayers)]

        dAp = []
        for i in range(2):
            t = perm.tile([128, SP * T], BF16, tag=f"dA{i}", name=f"dA{i}")
            nc.vector.memset(t[:, 0:SP * T:T], 0.0)
            dAp.append(t)

        bounceY = [nc.dram_tensor(f"bounceY{i}", [T, D], BF16)
                   for i in range(2)]
        xdbl_dram = nc.dram_tensor("xdbl_dram", [32, T], BF16)

        def brd(src_rows_ap, n):
            # broadcast n DRAM rows of length T to 128 partitions
            return src_rows_ap.rearrange("a b -> (a b)").partition_broadcast(128)

        for bi in range(nbatches):
            # ================ preamble (feature-major) ================
            a_fm = [load(bsl("a_fm", bi, rows=(c * 128, (c + 1) * 128)),
                         [128, T], f"a_fm{c}", BF16, pool=pwork)
                    for c in range(NDCH)]
            # a_tm[j-block layout]: time-major audio direct from blob
            a_tm = load(bsl("a_tmb", bi), [128, NTB * D], "a_tm", BF16)
            g_a2v = [pwork.tile([128, T], BF16, tag=f"gate{c}",
                                name=f"g_a2v{c}") for c in range(NDCH)]
            for ec in range(NDCH):
                for th in range(2):
                    ps = ptrans.tile([128, 512], F32, tag="ps", name="ps")
                    for dc in range(NDCH):
                        nc.tensor.matmul(ps[:],
                                         w_a2v[dc][:, ec * 128:(ec + 1) * 128],
                                         a_fm[dc][:, th_sl[th]],
                                         start=(dc == 0), stop=(dc == NDCH - 1))
                    nc.scalar.activation(g_a2v[ec][:, th_sl[th]], ps[:],
                                         AF.Sigmoid, bias=b_a2v[:, ec:ec + 1])
            v_fm = [load(bsl("v_fm", bi, rows=(c * 128, (c + 1) * 128)),
                         [128, T], f"a_fm{c}", BF16, pool=pwork)
                    for c in range(NDCH)]
            v_pad = [pwork.tile([128, T + 3], BF16, tag=f"xc_pad{c}",
                                name=f"v_pad{c}") for c in range(NDCH)]
            for c in range(NDCH):
                nc.vector.memset(v_pad[c][:, 0:1], 0.0)
                nc.vector.memset(v_pad[c][:, T + 1:T + 2], 0.0)
                nc.gpsimd.tensor_tensor(out=v_pad[c][:, 1:T + 1],
                                        in0=v_fm[c][:], in1=g_a2v[c][:],
                                        op=OP.mult)
            g_v2a = [pwork.tile([128, T], BF16, tag=f"gate{c+2}",
                                name=f"g_v2a{c}") for c in range(NDCH)]
            for ec in range(NDCH):
                for th in range(2):
                    ps = ptrans.tile([128, 512], F32, tag="ps", name="ps")
                    for dc in range(NDCH):
                        nc.tensor.matmul(ps[:],
                                         w_v2a[dc][:, ec * 128:(ec + 1) * 128],
                                         v_pad[dc][:, th * 512 + 1:
                                                    th * 512 + 513],
                                         start=(dc == 0), stop=(dc == NDCH - 1))
                    nc.scalar.activation(g_v2a[ec][:, th_sl[th]], ps[:],
                                         AF.Sigmoid, bias=b_v2a[:, ec:ec + 1])
            dlt = [pwork.tile([128, T], BF16, tag=f"xcs{c}", name=f"dlt{c}")
                   for c in range(NDCH)]
            for ec in range(NDCH):
                for th in range(2):
                    ps = ptrans.tile([128, 512], F32, tag="ps", name="ps")
                    first = True
                    for k in range(3):
                        for dc in range(NDCH):
                            nc.tensor.matmul(
                                ps[:], pk[k][dc][:, ec * 128:(ec + 1) * 128],
                                v_pad[dc][:, th * 512 + k: th * 512 + k + 512],
                                start=first, stop=(k == 2 and dc == NDCH - 1))
                            first = False
                    nc.scalar.activation(dlt[ec][:, th_sl[th]], ps[:],
                                         AF.Gelu, bias=bn_b[:, ec:ec + 1],
                                         scale=bn_s[:, ec:ec + 1])
            gdlt = [pwork.tile([128, T], BF16, tag=f"xcs{c+2}",
                               name=f"gdlt{c}") for c in range(NDCH)]
            for c in range(NDCH):
                nc.gpsimd.tensor_tensor(out=gdlt[c][:], in0=g_v2a[c][:],
                                        in1=dlt[c][:], op=OP.mult)
            # x0 (time-major f32) = a_tm + transpose(gdlt)
            gd_tm = pwork.tile([128, NTB * D], BF16, tag="h_tm",
                               name="gd_tm")
            for j in range(NTB):
                for dc in range(NDCH):
                    psT = ppst.tile([128, 128], BF16, tag="pst",
                                    name="pst")
                    nc.tensor.transpose(psT[:],
                                        gdlt[dc][:, j * 128:(j + 1) * 128],
                                        id128b[:])
                    nc.scalar.copy(
                        gd_tm[:, j * D + dc * 128: j * D + (dc + 1) * 128],
                        psT[:])
            x_tm = perm.tile([128, NTB * D], BF16, tag="x_tm0",
                             name="x_tm0")
            nc.vector.tensor_tensor(out=x_tm[:], in0=a_tm[:], in1=gd_tm[:],
                                    op=OP.add)

            # ================ mamba layers ================
            for l in range(nlayers):
                # rmsnorm over channel dim
                st = psmall.tile([128, NTB], F32, tag="st", name="st")
                sq = pwork.tile([128, D], F32, tag="sq", name="sq")
                for j in range(NTB):
                    nc.scalar.activation(sq[:], x_tm[:, j * D:(j + 1) * D],
                                         AF.Square, accum_out=st[:, j:j + 1])
                ms = psmall.tile([128, NTB], F32, tag="ms", name="ms")
                nc.vector.tensor_scalar(out=ms[:], in0=st[:],
                                        scalar1=1.0 / D, scalar2=1e-5,
                                        op0=OP.mult, op1=OP.add)
                msr = psmall.tile([128, NTB], F32, tag="msr", name="msr")
                nc.vector.reciprocal(out=msr[:], in_=ms[:])
                rstd = psmall.tile([128, NTB], F32, tag="rstd", name="rstd")
                nc.scalar.activation(rstd[:], msr[:], AF.Sqrt)
                h_tm = pwork.tile([128, NTB * D], BF16, tag="h_tm",
                                  name="h_tm")
                for j in range(NTB):
                    nc.vector.scalar_tensor_tensor(
                        out=h_tm[:, j * D:(j + 1) * D],
                        in0=x_tm[:, j * D:(j + 1) * D],
                        scalar=rstd[:, j:j + 1], in1=rmsw[l][:],
                        op0=OP.mult, op1=OP.mult)
                h_fm = [pwork.tile([128, T], BF16, tag=f"h_fm{c}",
                                   name=f"h_fm{c}") for c in range(NDCH)]
                for j in range(NTB):
                    for dc in range(NDCH):
                        psT = ppst.tile([128, 128], BF16, tag="pst",
                                        name="pst")
                        nc.tensor.transpose(
                            psT[:],
                            h_tm[:, j * D + dc * 128: j * D + (dc + 1) * 128],
                            id128b[:])
                        nc.scalar.copy(h_fm[dc][:, j * 128:(j + 1) * 128],
                                       psT[:])
                h_rev_fm = [h_fm[c][:, ::-1] for c in range(NDCH)]

                ytm = [None, None]
                x_new = None
                for r in range(2):
                    hsrc = h_fm if r == 0 else h_rev_fm
                    w_in_l = [load(bsl("w_in", l, r,
                                       rows=(c * 128, (c + 1) * 128)),
                                   [128, 2 * DI], f"w_in{c}", BF16, pool=pwt)
                              for c in range(NDCH)]
                    cwd_t = load(bsl("cwd", l, r),
                                 [128, NICH * DCONV * 128], "cwd", BF16,
                                 pool=pwt)
                    cwd_l = [[cwd_t[:, (c * DCONV + k) * 128:
                                    (c * DCONV + k + 1) * 128]
                              for k in range(DCONV)] for c in range(NICH)]
                    cb_l = load(bsl("cb", l, r), [128, NICH], "cb", pool=pwt)
                    w_xp_l = [load(bsl("w_xp", l, r,
                                       rows=(c * 128, (c + 1) * 128)),
                                   [128, 80], f"w_xp{c}", BF16, pool=pwt)
                              for c in range(NICH)]
                    w_dt_l = load(bsl("w_dt", l, r), [DTR, DI], "w_dt",
                                  BF16, pool=pwt)
                    dtb_l = load(bsl("dtb", l, r), [128, NICH], "dtb",
                                 pool=pwt)
                    a_neg_l = load(bsl("a_neg", l, r), [128, NICH * DS],
                                   "a_neg", pool=pwt)
                    dskd_t = load(bsl("dskd", l, r), [128, NICH * 128],
                                  "dskd", BF16, pool=pwt)
                    dskd_l = [dskd_t[:, c * 128:(c + 1) * 128]
                              for c in range(NICH)]
                    w_out_l = [load(bsl("w_out", l, r,
                                        rows=(c * 128, (c + 1) * 128)),
                                    [128, D], f"w_out{c}", BF16, pool=pwt)
                               for c in range(NICH)]

                    # ---- in_proj -> xc (padded bf16), silu(z) ----
                    xc_pad = [pwork.tile([128, T + 3], BF16,
                                         tag=f"xc_pad{c}", name=f"xc_pad{c}")
                              for c in range(NICH)]
                    g_z = [pwork.tile([128, T], BF16, tag=f"gate{c}",
                                      name=f"g_z{c}") for c in range(NICH)]
                    for c in range(NICH):
                        nc.vector.memset(xc_pad[c][:, 0:3], 0.0)
                    for ec in range(2 * NICH):
                        for th in range(2):
                            ps = ptrans.tile([128, 512], F32, tag="ps",
                                             name="ps")
                            for dc in range(NDCH):
                                nc.tensor.matmul(
                                    ps[:],
                                    w_in_l[dc][:, ec * 128:(ec + 1) * 128],
                                    hsrc[dc][:, th_sl[th]],
                                    start=(dc == 0), stop=(dc == NDCH - 1))
                            if ec < NICH:
                                nc.scalar.copy(
                                    xc_pad[ec][:, 3 + th * 512:
                                               3 + th * 512 + 512], ps[:])
                            else:
                                nc.scalar.activation(
                                    g_z[ec - NICH][:, th_sl[th]], ps[:],
                                    AF.Silu)
                    # ---- depthwise conv: 4 accumulating diag matmuls ----
                    xc_s = [pwork.tile([128, T], BF16, tag=f"xcs{c}",
                                       name=f"xc_s{c}") for c in range(NICH)]
                    for c in range(NICH):
                        for th in range(2):
                            psc = ptrans.tile([128, 512], F32, tag="ps",
                                              name="ps")
                            for k in range(DCONV):
                                nc.tensor.matmul(
                                    psc[:], cwd_l[c][k],
                                    xc_pad[c][:, th * 512 + k:
                                              th * 512 + k + 512],
                                    start=(k == 0), stop=(k == DCONV - 1))
                            nc.scalar.activation(xc_s[c][:, th_sl[th]],
                                                 psc[:], AF.Silu,
                                                 bias=cb_l[:, c:c + 1])
                    # ---- x_proj -> xdbl rows [dt(16) | B(16) | C(16)] ----
                    xdbl = pwork.tile([DTR, T], BF16, tag="xdbl",
                                      name="xdbl")
                    xdbl_bf = pwork.tile([48, T], BF16, tag="xdbl_bf",
                                         name="xdbl_bf")
                    for th in range(2):
                        psx = ptrans.tile([80, 512], F32, tag="ps",
                                          name="ps")
                        for c in range(NICH):
                            nc.tensor.matmul(psx[:], w_xp_l[c][:],
                                             xc_s[c][:, th_sl[th]],
                                             start=(c == 0),
                                             stop=(c == NICH - 1))
                        nc.scalar.copy(xdbl[0:DTR, th_sl[th]], psx[0:DTR, :])
                        nc.scalar.copy(xdbl_bf[0:16, th_sl[th]],
                                       psx[32:48, :])
                        nc.scalar.copy(xdbl_bf[32:48, th_sl[th]],
                                       psx[64:80, :])
                        nc.sync.dma_start(xdbl_dram[0:16, th_sl[th]],
                                          xdbl_bf[0:16, th_sl[th]])
                        nc.sync.dma_start(xdbl_dram[16:32, th_sl[th]],
                                          xdbl_bf[32:48, th_sl[th]])

                    # ---- prep: delta (bf16), u (bf16), pipelined 2 ahead
                    # (delta/u rotate on c%2 tags; prep(c+2) is emitted
                    # after chunk c's exps so the ACT queue stays acyclic)
                    deltas, us = [None] * NICH, [None] * NICH

                    def prep(c):
                        psd = [ptrans.tile([128, 512], F32, tag="ps",
                                           name="ps") for _ in range(2)]
                        for th in range(2):
                            nc.tensor.matmul(psd[th][:],
                                             w_dt_l[:, c * 128:(c + 1) * 128],
                                             xdbl[0:DTR, th_sl[th]],
                                             start=True, stop=True)
                        esp = pwork.tile([128, T], BF16, tag="esp",
                                         name="esp")
                        for th in range(2):
                            nc.scalar.activation(esp[:, th_sl[th]],
                                                 psd[th][:], AF.Exp,
                                                 bias=dtb_l[:, c:c + 1])
                        delta = pwork.tile([128, T], BF16,
                                           tag=f"delta{c % 2}",
                                           name=f"delta{c % 2}")
                        nc.scalar.activation(delta[:], esp[:], AF.Ln,
                                             bias=1.0)
                        deltas[c] = delta
                        u = pwork.tile([128, T], BF16, tag=f"u{c % 2}",
                                       name=f"u{c % 2}")
                        nc.vector.tensor_tensor(out=u[:], in0=delta[:],
                                                in1=xc_s[c][:], op=OP.mult)
                        us[c] = u

                    prep(0)
                    prep(1)

                    # ---- scan section ----
                    y_g = [pwork.tile([128, T], BF16, tag=f"y_g{c}",
                                      name=f"y_g{c}") for c in range(NICH)]
                    for c in range(NICH):
                        psy = [ppsy.tile([128, 512], F32, tag=f"psy{th}",
                                         name=f"psy{th}") for th in range(2)]
                        for g in range(NG):
                            s0 = g * SP
                            dA = dAp[g % 2]
                            if "no_exp" not in probe:
                                for sv in range(SP):
                                    nc.scalar.activation(
                                        dA[:, sv * T + 1:(sv + 1) * T],
                                        deltas[c][:, 1:T], AF.Exp,
                                        scale=a_neg_l[:, c * DS + s0 + sv:
                                                      c * DS + s0 + sv + 1])
                            bm = pbc.tile([128, SP * T], BF16, tag="bm",
                                          name="bm")
                            nc.sync.dma_start(
                                bm[:], brd(xdbl_dram[s0:s0 + SP, :], SP))
                            cm = pbc.tile([128, SP * T], BF16, tag="cm",
                                          name="cm")
                            nc.sync.dma_start(
                                cm[:], brd(xdbl_dram[16 + s0:16 + s0 + SP, :],
                                           SP))
                            dBu = pdbu.tile([128, SP * T], BF16, tag="dBu",
                                            name="dBu")
                            if "no_dbu" not in probe:
                                nc.vector.tensor_tensor(
                                out=dBu[:].rearrange("p (a t) -> p a t",
                                                     a=SP),
                                in0=us[c][:].rearrange("p (a t) -> p a t",
                                                       a=1)
                                .broadcast_to([128, SP, T]),
                                    in1=bm[:].rearrange("p (a t) -> p a t",
                                                        a=SP),
                                    op=OP.mult)
                            hsc = pscan.tile([128, SP * T], BF16, tag="hsc",
                                             name="hsc")
                            if "scan_copy" in probe:
                                nc.vector.tensor_copy(out=hsc[:],
                                                      in_=dBu[:])
                            else:
                                nc.vector.tensor_tensor_scan(
                                    hsc[:], dA[:], dBu[:], 0.0,
                                    OP.mult, OP.add)
                            hc = pbc.tile([128, SP * T], BF16, tag="hc",
                                          name="hc")
                            heng = nc.gpsimd if "hc_gps" in probe \
                                else nc.vector
                            heng.tensor_tensor(out=hc[:], in0=hsc[:],
                                               in1=cm[:], op=OP.mult)
                            for sv in range(SP):
                                if "no_psy" in probe and not (g == 0 and sv == 0):
                                    continue
                                for th in range(2):
                                    nc.tensor.matmul(
                                        psy[th][:], id128b[:],
                                        hc[:, sv * T + th * 512:
                                           sv * T + th * 512 + 512],
                                        start=(g == 0 and sv == 0),
                                        stop=False)
                        for th in range(2):
                            nc.tensor.matmul(psy[th][:], dskd_l[c],
                                             xc_s[c][:, th_sl[th]],
                                             start=False, stop=True)
                        for th in range(2):
                            nc.vector.tensor_tensor(out=y_g[c][:, th_sl[th]],
                                                    in0=psy[th][:],
                                                    in1=g_z[c][:, th_sl[th]],
                                                    op=OP.mult)
                        if c + 2 < NICH:
                            prep(c + 2)

                    # ---- out_proj (time-major j-blocks); fwd residual
                    # add folds straight out of PSUM on DVE ----
                    if r == 0:
                        x_new = perm.tile([128, NTB * D], BF16,
                                          tag=f"x_tm{(l + 1) % 2}",
                                          name=f"x_tm{(l + 1) % 2}")
                    else:
                        ytm[1] = pwork.tile([128, NTB * D], BF16,
                                            tag="ytm1", name="ytm1")
                    for jh in range(2):
                        pso = ppso.tile([128, 4 * D], F32, tag="pso",
                                        name="pso")
                        for j4 in range(4):
                            j = jh * 4 + j4
                            for c in range(NICH):
                                nc.tensor.matmul(
                                    pso[:, j4 * D:(j4 + 1) * D],
                                    y_g[c][:, j * 128:(j + 1) * 128],
                                    w_out_l[c][:],
                                    start=(c == 0), stop=(c == NICH - 1))
                        if r == 0:
                            nc.vector.tensor_tensor(
                                out=x_new[:, jh * 4 * D:(jh + 1) * 4 * D],
                                in0=pso[:],
                                in1=x_tm[:, jh * 4 * D:(jh + 1) * 4 * D],
                                op=OP.add)
                        else:
                            nc.scalar.copy(
                                ytm[1][:, jh * 4 * D:(jh + 1) * 4 * D],
                                pso[:])

                # reverse bwd output on-chip: anti-diagonal permutation
                # matmul flips partitions; j-blocks are read mirrored
                artm = pwork.tile([128, NTB * D], BF16, tag="artm",
                                  name="artm")
                for j in range(NTB):
                    psR = ptrans.tile([128, D], F32, tag="ps", name="ps")
                    nc.tensor.matmul(
                        psR[:], rev128b[:],
                        ytm[1][:, (NTB - 1 - j) * D:(NTB - j) * D],
                        start=True, stop=True)
                    nc.scalar.copy(artm[:, j * D:(j + 1) * D], psR[:])
                nc.vector.tensor_tensor(out=x_new[:], in0=x_new[:],
                                        in1=artm[:], op=OP.add)
                x_tm = x_new

            # ================ final channel LayerNorm ================
            s_t = pwork.tile([128, NTB * D], BF16, tag="h_tm", name="s_t")
            nc.vector.tensor_tensor(out=s_t[:], in0=x_tm[:], in1=a_tm[:],
                                    op=OP.add)
            stm = psmall.tile([128, NTB], F32, tag="stm", name="stm")
            stv = psmall.tile([128, NTB], F32, tag="stv", name="stv")
            dump = pwork.tile([128, D], F32, tag="sq", name="dump")
            for j in range(NTB):
                nc.scalar.activation(dump[:], s_t[:, j * D:(j + 1) * D],
                                     AF.Copy, accum_out=stm[:, j:j + 1])
                nc.scalar.activation(dump[:], s_t[:, j * D:(j + 1) * D],
                                     AF.Square, accum_out=stv[:, j:j + 1])
            mu = psmall.tile([128, NTB], F32, tag="mu", name="mu")
            nc.vector.tensor_scalar(out=mu[:], in0=stm[:], scalar1=1.0 / D,
                                    scalar2=None, op0=OP.mult)
            var = psmall.tile([128, NTB], F32, tag="var", name="var")
            nc.vector.tensor_scalar(out=var[:], in0=stv[:], scalar1=1.0 / D,
                                    scalar2=None, op0=OP.mult)
            mu2 = psmall.tile([128, NTB], F32, tag="mu2", name="mu2")
            nc.vector.tensor_tensor(out=mu2[:], in0=mu[:], in1=mu[:],
                                    op=OP.mult)
            nc.vector.tensor_tensor(out=var[:], in0=var[:], in1=mu2[:],
                                    op=OP.subtract)
            ve = psmall.tile([128, NTB], F32, tag="ve", name="ve")
            nc.vector.tensor_scalar(out=ve[:], in0=var[:], scalar1=EPS,
                                    scalar2=None, op0=OP.add)
            vr = psmall.tile([128, NTB], F32, tag="vr", name="vr")
            nc.vector.reciprocal(out=vr[:], in_=ve[:])
            rstd2 = psmall.tile([128, NTB], F32, tag="rstd2", name="rstd2")
            nc.scalar.activation(rstd2[:], vr[:], AF.Sqrt)
            otm = pwork.tile([128, NTB * D], BF16, tag="gd_out", name="otm")
            for j in range(NTB):
                sl = slice(j * D, (j + 1) * D)
                nc.vector.tensor_scalar(out=otm[:, sl], in0=s_t[:, sl],
                                        scalar1=mu[:, j:j + 1],
                                        scalar2=rstd2[:, j:j + 1],
                                        op0=OP.subtract, op1=OP.mult)
            otm3 = otm[:].rearrange("p (j d) -> p j d", j=NTB)
            nc.vector.tensor_tensor(out=otm3, in0=otm3,
                                    in1=g_bc[:].rearrange(
                                        "p (a d) -> p a d", a=1)
                                    .broadcast_to([128, NTB, D]),
                                    op=OP.mult)
            nc.vector.tensor_tensor(out=otm3, in0=otm3,
                                    in1=be_bc[:].rearrange(
                                        "p (a d) -> p a d", a=1)
                                    .broadcast_to([128, NTB, D]),
                                    op=OP.add)
            nc.sync.dma_start(
                out_d[bi * T:(bi + 1) * T, :]
                .rearrange("(j p) d -> p j d", p=128),
                otm[:].rearrange("p (j d) -> p j d", j=NTB))


# ---------------- host side ----------------

def make_in_maps(inputs, num_cores=None):
    # one map per core; core i owns batch i (num_cores arg kept for
    # interface compatibility, the mesh is always N_CORES wide)
    inp = {k: np.asarray(v, dtype=np.float32) for k, v in inputs.items()}
    m = {}
    m["a_fm"] = np.ascontiguousarray(inp["audio"].transpose(0, 2, 1))
    # time-major j-block layout: a_tmb[b, p, j*D + d] = audio[b, j*128+p, d]
    m["a_tmb"] = np.ascontiguousarray(
        inp["audio"].reshape(B, NTB, 128, D).transpose(0, 2, 1, 3)
        .reshape(B, 128, NTB * D))
    m["v_fm"] = np.ascontiguousarray(inp["video"].transpose(0, 2, 1))
    m["w_a2v"] = np.ascontiguousarray(inp["gate_a2v_w"].T)
    m["b_a2v"] = np.ascontiguousarray(inp["gate_a2v_b"].reshape(NDCH, 128).T)
    m["w_v2a"] = np.ascontiguousarray(inp["gate_v2a_w"].T)
    m["b_v2a"] = np.ascontiguousarray(inp["gate_v2a_b"].reshape(NDCH, 128).T)
    m["pk"] = np.ascontiguousarray(
        np.stack([inp["proj_w"][:, :, k].T for k in range(3)]))
    m["bn_s"] = np.ascontiguousarray(
        (inp["bn_gamma"] / np.sqrt(1.0 + 1e-5)).reshape(NDCH, 128).T)
    m["bn_b"] = np.ascontiguousarray(inp["bn_beta"].reshape(NDCH, 128).T)
    m["rmsw_bc"] = np.ascontiguousarray(
        np.broadcast_to(inp["rms_w"][:, None, :], (NM, 128, D)))
    m["w_in"] = np.ascontiguousarray(inp["in_proj_w"].transpose(0, 1, 3, 2))
    # conv taps as diagonal matrices: cwd[l,r,c,k] = diag(conv_w[l,r,c*128:(c+1)*128,k])
    cw = inp["conv_w"]  # [NM,2,DI,DCONV]
    cwd = np.zeros((NM, 2, 128, NICH * DCONV * 128), np.float32)
    ii = np.arange(128)
    for l in range(NM):
        for r in range(2):
            for c in range(NICH):
                for k in range(DCONV):
                    cwd[l, r, ii, (c * DCONV + k) * 128 + ii] = \
                        cw[l, r, c * 128:(c + 1) * 128, k]
    m["cwd"] = cwd
    m["cb"] = np.ascontiguousarray(
        inp["conv_b"].reshape(NM, 2, NICH, 128).transpose(0, 1, 3, 2))
    w_xp_p = np.zeros((NM, 2, DI, 80), np.float32)
    for l in range(NM):
        for r in range(2):
            xp_t = inp["x_proj_w"][l, r].T  # [DI, 48]
            w_xp_p[l, r, :, 0:DTR] = xp_t[:, 0:DTR]
            w_xp_p[l, r, :, 32:32 + DS] = xp_t[:, DTR:DTR + DS]
            w_xp_p[l, r, :, 64:64 + DS] = xp_t[:, DTR + DS:DTR + 2 * DS]
    m["w_xp"] = w_xp_p
    m["w_dt"] = np.ascontiguousarray(inp["dt_w"].transpose(0, 1, 3, 2))
    m["dtb"] = np.ascontiguousarray(
        inp["dt_b"].reshape(NM, 2, NICH, 128).transpose(0, 1, 3, 2))
    m["a_neg"] = np.ascontiguousarray(
        (-np.exp(inp["A_log"])).reshape(NM, 2, NICH, 128, DS)
        .transpose(0, 1, 3, 2, 4).reshape(NM, 2, 128, NICH * DS))
    dsk = inp["D_skip"]  # [NM,2,DI]
    dskd = np.zeros((NM, 2, 128, NICH * 128), np.float32)
    for l in range(NM):
        for r in range(2):
            for c in range(NICH):
                dskd[l, r, ii, c * 128 + ii] = dsk[l, r, c * 128:(c + 1) * 128]
    m["dskd"] = dskd
    m["w_out"] = np.ascontiguousarray(inp["out_w"].transpose(0, 1, 3, 2))
    m["g_bc"] = np.ascontiguousarray(np.broadcast_to(inp["cln_gamma"],
                                                     (128, D)))
    m["be_bc"] = np.ascontiguousarray(np.broadcast_to(inp["cln_beta"],
                                                      (128, D)))
    m["id128"] = np.eye(128, dtype=np.float32)
    m["rev128"] = np.eye(128, dtype=np.float32)[::-1].copy()
    t_of = np.arange(T, dtype=np.int64).reshape(NTB, 128).T  # [128, NTB]
    m["idx"] = np.ascontiguousarray(((T - 1) - t_of).astype(np.int32))

    import ml_dtypes
    a_fm_all, v_fm_all = m["a_fm"], m["v_fm"]
    a_tmb_all = m["a_tmb"]
    blobs = []
    for ci in range(N_CORES):
        m["a_fm"] = a_fm_all[ci:ci + 1]
        m["v_fm"] = v_fm_all[ci:ci + 1]
        m["a_tmb"] = a_tmb_all[ci:ci + 1]
        parts = []
        for name, shape, dt in PACK:
            a = np.ascontiguousarray(m[name])
            assert a.shape == tuple(shape), (name, a.shape, shape)
            if dt == "h":
                a = a.astype(ml_dtypes.bfloat16).ravel().view(np.float32)
            elif dt == "i":
                a = a.view(np.float32).ravel()
            else:
                a = a.ravel()
            parts.append(a)
        blobs.append({"blob": np.concatenate(parts)})
    return blobs


def get_nc(**kw):
    key = ("nc", tuple(sorted(kw.items())))
    if key not in _CACHE:
        _CACHE[key] = build_nc(**kw)
    return _CACHE[key]


def kernel(**inputs) -> np.ndarray:
    nc = get_nc()
    maps = make_in_maps(inputs)
    res = run_bass_kernel_spmd(nc, maps, list(range(N_CORES)))
    out = np.stack([res.results[i]["out"].reshape(T, D)
                    for i in range(N_CORES)])
    return out.astype(np.float32)


if __name__ == "__main__":
    import reference
    inputs = {k: np.asarray(v) for k, v in reference.setup_inputs().items()}
    got = kernel(**inputs)
    print("kernel ran; out shape", got.shape)
